# revision 1
# baseline (speedup 1.0000x reference)
"""nn_CrossAttention kernel v2 for 8x TRN2 NeuronCores.

Sharding: core c = (batch b = c//2, head-group hg = c%2 of 8 heads).

v2 design (vs baseline): goal is a gap-free tensor-engine stream so the
PE clock ramps to max and stays there.
 - wq/wk/wv fully SBUF-resident (loaded once).
 - qT/kT stored bf16 (halves SBUF + QK LDW time).
 - Fine-grained interleave: QK i-slot of phase k + AV i-slot of phase
   k-1, so PE always outpaces ACT (exp) per slot.
 - Normalization has no PE instructions: DVE reciprocal of the
   denominator row, DRAM-bounce broadcast DMA to 64 partitions, DVE
   multiply. Emitted at end of the consuming phase; PE never waits on
   DVE.
 - Projections for p=2,3 spread across attention phases 1..7.
 - PSUM: vp 2 + sc 3 + av 3 = 8 banks.
"""

import json
import numpy as np

B, S, D, NH, HD = 4, 2048, 1024, 16, 64
CPC = 512          # cols per core = 8 heads * 64
NCORES = 8
NDT = D // 128     # 8 d-tiles
NP = CPC // 128    # 4 c-tiles (head pairs)
NSK = S // 128     # 16 sk-tiles
NJ = S // 512      # 4 sq chunks
SQC = 512          # sq chunk size
NPH = NP * NJ      # 16 phases



# ---------------------------------------------------------------- drain fix
def _fix_module_json(bj: bytes) -> bytes:
    """This walrus build accepts at most ONE sync wait/update on CTRL-lowered
    instructions (Drain). Move extras onto EventSemaphore instructions."""
    d = json.loads(bj)
    counter = [0]

    def fix_block(b):
        out = []
        for inst in b.get("instructions", []):
            si = inst.get("sync_info") or {}
            ow = si.get("on_wait") or []
            ou = si.get("on_update") or []
            if (inst.get("opcode") not in
                    ("EventSemaphore", "Call", "RegisterMove",
                     "UnconditionalBranch", "ISA", "Drain") and len(ow) > 1):
                for w in ow[1:]:
                    counter[0] += 1
                    out.append({
                        "debug": inst.get("debug", 0),
                        "engine": inst["engine"],
                        "ins": [], "outs": [],
                        "name": f"synthmmw-{counter[0]}",
                        "opcode": "EventSemaphore",
                        "sync_info": {"on_update": [], "on_wait": [w]},
                    })
                inst["sync_info"] = {"on_update": ou, "on_wait": ow[:1]}
                out.append(inst)
                continue
            if inst.get("opcode") == "Drain" and (len(ow) > 1 or len(ou) > 1):
                for w in ow[1:]:
                    counter[0] += 1
                    out.append({
                        "debug": inst.get("debug", 0),
                        "engine": inst["engine"],
                        "ins": [], "outs": [],
                        "name": f"synthwait-{counter[0]}",
                        "opcode": "EventSemaphore",
                        "sync_info": {"on_update": [], "on_wait": [w]},
                    })
                inst["sync_info"] = {"on_update": ou[:1], "on_wait": ow[:1]}
                out.append(inst)
                for u in ou[1:]:
                    counter[0] += 1
                    out.append({
                        "debug": inst.get("debug", 0),
                        "engine": inst["engine"],
                        "ins": [], "outs": [],
                        "name": f"synthupd-{counter[0]}",
                        "opcode": "EventSemaphore",
                        "sync_info": {"on_update": [u], "on_wait": []},
                    })
            else:
                out.append(inst)
        b["instructions"] = out
        for sb in b.get("blocks", []):
            fix_block(sb)

    for fn in d.get("functions", []):
        for blk in fn.get("blocks", []):
            fix_block(blk)
    return json.dumps(d).encode()


def _install_drainfix():
    import concourse.bass as bass
    if getattr(bass.Bass, "_drainfix_installed", False):
        return
    orig = bass.Bass.to_json_bytes

    def patched(self):
        return _fix_module_json(orig(self))

    bass.Bass.to_json_bytes = patched
    bass.Bass._drainfix_installed = True


# ---------------------------------------------------------------- program
def _build_nc(reps=1):
    import concourse.bass as bass
    import concourse.mybir as mybir
    from concourse.tile import TileContext
    from contextlib import ExitStack

    f32 = mybir.dt.float32
    f32r = mybir.dt.float32r
    bf16 = mybir.dt.bfloat16
    EXP = mybir.ActivationFunctionType.Exp

    nc = bass.Bass("TRN2", num_devices=NCORES)

    xqT = nc.dram_tensor("xqT", [D, S], f32, kind="ExternalInput")
    xkT = nc.dram_tensor("xkT", [D, S], f32, kind="ExternalInput")
    xvT = nc.dram_tensor("xvT", [D, S], f32, kind="ExternalInput")
    wq = nc.dram_tensor("wq", [D, CPC], f32, kind="ExternalInput")
    wk = nc.dram_tensor("wk", [D, CPC], f32, kind="ExternalInput")
    wv = nc.dram_tensor("wv", [D, CPC], f32, kind="ExternalInput")
    bqd = nc.dram_tensor("bq", [CPC], f32, kind="ExternalInput")
    bkd = nc.dram_tensor("bk", [CPC], f32, kind="ExternalInput")
    bvd = nc.dram_tensor("bv", [CPC], f32, kind="ExternalInput")
    outd = nc.dram_tensor("out", [CPC, S], f32, kind="ExternalOutput")
    # scratch for the denominator-reciprocal broadcast bounce (stride-0
    # partition reads are only legal from DRAM)
    recscr = nc.dram_tensor("recscr", [2 * NPH, SQC], f32, kind="Internal")

    with ExitStack() as ctx:
        ctx.enter_context(nc.allow_low_precision(
            reason="qk in bf16, matmul accumulates f32; rel-err gate 2e-2"))
        tc = ctx.enter_context(TileContext(nc))
        sb = ctx.enter_context(tc.tile_pool(name="sb", bufs=1))
        ps = ctx.enter_context(tc.tile_pool(name="ps", bufs=1, space="PSUM"))

        # ---- resident weights / constants (wk + first x first: critical) ----
        wk_sb = sb.tile([128, NDT, CPC], f32r, name="wk_sb")
        for dd in range(NDT):
            nc.sync.dma_start(out=wk_sb[:, dd, :],
                              in_=wk[dd * 128:(dd + 1) * 128, :].bitcast(f32r))
        bk_sb = sb.tile([128, NP], f32, name="bk_sb")
        nc.sync.dma_start(out=bk_sb, in_=bkd.rearrange("(p c) -> c p", p=NP))
        bq_sb = sb.tile([128, NP], f32, name="bq_sb")
        nc.sync.dma_start(out=bq_sb, in_=bqd.rearrange("(p c) -> c p", p=NP))

        # persistent activation tiles
        qT = [sb.tile([128, S], bf16, name=f"qT{p}") for p in range(NP)]
        kT = [sb.tile([128, S], bf16, name=f"kT{p}") for p in range(NP)]
        v_aug = sb.tile([128, NSK, 8, 65], bf16, name="v_aug")

        # ---- helpers ----
        def load_x(x_dram, t):
            xts = []
            for dd in range(NDT):
                x_t = sb.tile([128, SQC], f32r, tag="xs", bufs=12,
                              name=f"x_{t}_{dd}")
                nc.sync.dma_start(
                    out=x_t,
                    in_=x_dram[dd * 128:(dd + 1) * 128,
                               t * SQC:(t + 1) * SQC].bitcast(f32r))
                xts.append(x_t)
            return xts

        def emit_proj(t, plist, w_sb, x_dram, bias_sb, dst):
            """dst[p][:, t*512:(t+1)*512] = (x @ W + b).T chunk (bf16)."""
            xts = load_x(x_dram, t)
            for p in plist:
                pr = ps.tile([128, SQC], f32, tag="vp", bufs=2,
                             name=f"prj_{t}_{p}")
                for dd in range(NDT):
                    nc.tensor.matmul(
                        pr[:, :],
                        w_sb[:, dd, p * 128:(p + 1) * 128],
                        xts[dd][:, :],
                        start=(dd == 0), stop=(dd == NDT - 1))
                nc.vector.tensor_scalar_add(
                    dst[p][:, t * SQC:(t + 1) * SQC], pr[:, :],
                    bias_sb[:, p:p + 1])

        def emit_proj_v(tt):
            """v_aug[:, tt, h, 0:64] = (xv @ Wv + bv) rows tt*128.., bf16."""
            xvt = []
            for dd in range(NDT):
                xv_t = sb.tile([128, 128], f32r, tag="xv", bufs=10,
                               name=f"xv_{tt}_{dd}")
                nc.sync.dma_start(
                    out=xv_t,
                    in_=xvT[dd * 128:(dd + 1) * 128,
                            tt * 128:(tt + 1) * 128].bitcast(f32r))
                xvt.append(xv_t)
            pv = ps.tile([128, CPC], f32, tag="vp", bufs=2, name=f"pv_{tt}")
            for dd in range(NDT):
                nc.tensor.matmul(
                    pv[:, :], xvt[dd][:, :], wv_sb[:, dd, :],
                    start=(dd == 0), stop=(dd == NDT - 1))
            nc.vector.tensor_add(
                v_aug[:, tt, :, 0:64],
                pv.rearrange("c (h d) -> c h d", h=8),
                bv_bc.rearrange("c (h d) -> c h d", h=8))

        alpha = {}     # (c, h, i) -> [128, 512] bf16
        av_tiles = {}  # c -> {h: psum tile [65, 512]}

        def emit_qk_i(c, i):
            p, j = divmod(c, NJ)
            for h in range(2):
                sc = ps.tile([128, SQC], f32, tag="sc", bufs=3,
                             name=f"sc_{c}_{i}_{h}")
                nc.tensor.matmul(
                    sc[:, :],
                    kT[p][h * 64:(h + 1) * 64, i * 128:(i + 1) * 128],
                    qT[p][h * 64:(h + 1) * 64, j * SQC:(j + 1) * SQC],
                    start=True, stop=True)
                a_t = sb.tile([128, SQC], bf16, tag="alpha", bufs=36,
                              name=f"al_{c}_{i}_{h}")
                nc.scalar.activation(a_t[:, :], sc[:, :], EXP, scale=0.125)
                alpha[(c, h, i)] = a_t

        def emit_av_i(c, i):
            p, j = divmod(c, NJ)
            if i == 0:
                av_tiles[c] = {
                    h: ps.tile([65, SQC], f32, tag="av", bufs=3,
                               name=f"av_{c}_{h}")
                    for h in range(2)}
            for h in range(2):
                a_t = alpha.pop((c, h, i))
                nc.tensor.matmul(
                    av_tiles[c][h][:, :],
                    v_aug[:, i, 2 * p + h, :],
                    a_t[:, :],
                    start=(i == 0), stop=(i == NSK - 1))

        def emit_norm(c):
            """Normalize + write out chunk c. No PE instructions: DVE
            reciprocal of the denominator row, DRAM-bounce broadcast to 64
            partitions, DVE multiply. av psum tiles are freed by the DVE
            reads."""
            p, j = divmod(c, NJ)
            avt = av_tiles.pop(c)
            for h in range(2):
                r0 = (2 * p + h) * 64
                slot = 2 * c + h
                rec = sb.tile([1, SQC], f32, tag="rec", bufs=4,
                              name=f"rec_{c}_{h}")
                nc.vector.reciprocal(rec[:, :], avt[h][64:65, :])
                nc.sync.dma_start(out=recscr[slot:slot + 1, :],
                                  in_=rec[:, :])
                recB = sb.tile([64, SQC], f32, tag="recB", bufs=4,
                               name=f"recB_{c}_{h}")
                _r = recscr[slot:slot + 1, :]
                nc.sync.dma_start(
                    out=recB,
                    in_=bass.AP(tensor=_r.tensor, offset=_r.offset,
                                ap=[[0, 64]] + list(_r.ap)[1:]))
                cx = sb.tile([64, SQC], f32, tag="cx", bufs=4,
                             name=f"cx_{c}_{h}")
                nc.vector.tensor_mul(cx[:, :], avt[h][0:64, :], recB[:, :])
                nc.sync.dma_start(
                    out=outd[r0:r0 + 64, j * SQC:(j + 1) * SQC],
                    in_=cx[:, :])

        # ---- emission schedule ----
        def _emit_all():
            # prologue: kT then qT for p=0,1 (phases 0..7 cover p=0,1)
            for t in range(NJ):
                emit_proj(t, [0, 1], wk_sb, xkT, bk_sb, kT)
                if t == 0:
                    # wq/wv/bv DMAs fire while kT proj computes; their data
                    # is first needed one pass (wq) / two passes (wv) later
                    _emit_late_consts()
            for t in range(NJ):
                emit_proj(t, [0, 1], wq_sb, xqT, bq_sb, qT)

            # phase 0: QK(0) alone, then v projection (covers exp(0) on ACT)
            for i in range(NSK):
                emit_qk_i(0, i)
            for tt in range(NSK):
                emit_proj_v(tt)

            # pass-2 projection groups spread across phases 1..7
            pgroups = ([("k", t) for t in range(NJ)] +
                       [("q", t) for t in range(NJ)])
            # phase -> list of group indices
            sched = {1: [0, 1], 2: [2], 3: [3], 4: [4], 5: [5], 6: [6],
                     7: [7]}

            def emit_group(gi):
                kind, t = pgroups[gi]
                if kind == "k":
                    emit_proj(t, [2, 3], wk_sb, xkT, bk_sb, kT)
                else:
                    emit_proj(t, [2, 3], wq_sb, xqT, bq_sb, qT)

            for c in range(1, NPH):
                groups = list(sched.get(c, []))
                for i in range(NSK):
                    emit_qk_i(c, i)
                    emit_av_i(c - 1, i)
                    if i == 5 and groups:
                        emit_group(groups.pop(0))
                    if i == 11 and groups:
                        emit_group(groups.pop(0))
                emit_norm(c - 1)

            # epilogue
            for i in range(NSK):
                emit_av_i(NPH - 1, i)
            emit_norm(NPH - 1)

        # late-loaded residents (emitted after critical-path DMAs above,
        # but data only needed from mid-prologue onwards)
        wq_sb = sb.tile([128, NDT, CPC], f32r, name="wq_sb")
        wv_sb = sb.tile([128, NDT, CPC], f32r, name="wv_sb")
        bv_bc = sb.tile([128, CPC], f32, name="bv_bc")

        def _emit_late_consts():
            for dd in range(NDT):
                nc.sync.dma_start(
                    out=wq_sb[:, dd, :],
                    in_=wq[dd * 128:(dd + 1) * 128, :].bitcast(f32r))
            for dd in range(NDT):
                nc.sync.dma_start(
                    out=wv_sb[:, dd, :],
                    in_=wv[dd * 128:(dd + 1) * 128, :].bitcast(f32r))
            _bva = bvd[:]
            nc.sync.dma_start(
                out=bv_bc,
                in_=bass.AP(tensor=_bva.tensor, offset=_bva.offset,
                            ap=[[0, 128]] + list(_bva.ap)))
            nc.gpsimd.memset(v_aug[:, :, :, 64:65], 1.0)

        for _rep in range(reps):
            _emit_all()

    return nc


_NC_BY_REPS = {}


def _get_nc(reps=1):
    if reps not in _NC_BY_REPS:
        _install_drainfix()
        _NC_BY_REPS[reps] = _build_nc(reps)
    return _NC_BY_REPS[reps]


# ---------------------------------------------------------------- entry
def kernel(query, key_in, value, Wq, bq, Wk, bk, Wv, bv):
    from concourse.bass_utils import run_bass_kernel_spmd

    nc = _get_nc()
    query = np.asarray(query, np.float32)
    key_in = np.asarray(key_in, np.float32)
    value = np.asarray(value, np.float32)
    Wq = np.asarray(Wq, np.float32)
    Wk = np.asarray(Wk, np.float32)
    Wv = np.asarray(Wv, np.float32)
    bq = np.asarray(bq, np.float32)
    bk = np.asarray(bk, np.float32)
    bv = np.asarray(bv, np.float32)

    in_maps = []
    for c in range(NCORES):
        b, hg = divmod(c, 2)
        cols = slice(hg * CPC, (hg + 1) * CPC)
        in_maps.append({
            "xqT": np.ascontiguousarray(query[b].T),
            "xkT": np.ascontiguousarray(key_in[b].T),
            "xvT": np.ascontiguousarray(value[b].T),
            "wq": np.ascontiguousarray(Wq[:, cols]),
            "wk": np.ascontiguousarray(Wk[:, cols]),
            "wv": np.ascontiguousarray(Wv[:, cols]),
            "bq": np.ascontiguousarray(bq[cols]),
            "bk": np.ascontiguousarray(bk[cols]),
            "bv": np.ascontiguousarray(bv[cols]),
        })

    res = run_bass_kernel_spmd(nc, in_maps, core_ids=list(range(NCORES)))

    out = np.empty((B, S, D), np.float32)
    for c in range(NCORES):
        b, hg = divmod(c, 2)
        out[b, :, hg * CPC:(hg + 1) * CPC] = res.results[c]["out"].T
    return out



# revision 5
# speedup vs baseline: 1.2641x; 1.2641x over previous
"""nn_CrossAttention kernel v3 for 8x TRN2 NeuronCores.

Sharding: core c = (batch b = c//2, head-group hg = c%2 of 8 heads).

v3 design (vs v2 baseline at 746us):
 - Root cause of v2: PE stuck at HAM K=4/8 (1.2 GHz) for the last 535us.
   Steady state was ACT(exp)-paced (~21.5us/phase vs 13.6us warm PE), so
   the PE micro-idled, got re-throttled, and a saturated-cold PE never
   recovers. Fix: make the PE the pacer everywhere.
 - exp offload: per phase, 20 slots on ACT (exact exp), 12 on DVE via a
   Schraudolph-style bit-trick exp that writes the bf16 bit pattern as
   int16: bits = round(score*A + B). ~1.8% rms on those slots; fraction
   keeps total rel err ~1.2e-2 < 2e-2 gate. (GPSIMD has no PSUM access,
   so the Pool engine cannot help here.)
 - Single-pass prologue: all 4 head-pair column tiles per t-pass, so
   xq/xk are loaded ONCE (saves 16MB DMA vs v2's two passes).
 - Attention phases are pure QK/AV (PE-dense, no proj groups).
 - v-projection interleaved with phase 1 (xv DMA paced).
 - PSUM: vp 2 + sc 3 + av 3 = 8 banks.
"""

import json
import numpy as np

B, S, D, NH, HD = 4, 2048, 1024, 16, 64
CPC = 512          # cols per core = 8 heads * 64
NCORES = 8
NDT = D // 128     # 8 d-tiles
NP = CPC // 128    # 4 c-tiles (head pairs)
NSK = S // 128     # 16 sk-tiles
NJ = S // 512      # 4 sq chunks
SQC = 512          # sq chunk size
NPH = NP * NJ      # 16 phases

# bit-trick exp constants: bf16 bits = round(score * EXPA + EXPB)
# exp(s*0.125) = 2^(s*0.125*log2 e); bf16 bits = exp_field*128 + mantissa
EXPA = 0.125 * 1.4426950408889634 * 128.0   # 23.0831...
EXPB = 16256.0 - 7.5                        # 127*128 - sigma (sigma tuned)

# slot -> engine assignment (m = (2*i+h) % 16)
DVE_SLOTS = frozenset((1, 3, 6, 9, 11, 14))


# ---------------------------------------------------------------- drain fix
def _fix_module_json(bj: bytes) -> bytes:
    """This walrus build accepts at most ONE sync wait/update on CTRL-lowered
    instructions (Drain). Move extras onto EventSemaphore instructions."""
    d = json.loads(bj)
    counter = [0]

    def fix_block(b):
        out = []
        for inst in b.get("instructions", []):
            si = inst.get("sync_info") or {}
            ow = si.get("on_wait") or []
            ou = si.get("on_update") or []
            if (inst.get("opcode") not in
                    ("EventSemaphore", "Call", "RegisterMove",
                     "UnconditionalBranch", "ISA", "Drain") and len(ow) > 1):
                for w in ow[1:]:
                    counter[0] += 1
                    out.append({
                        "debug": inst.get("debug", 0),
                        "engine": inst["engine"],
                        "ins": [], "outs": [],
                        "name": f"synthmmw-{counter[0]}",
                        "opcode": "EventSemaphore",
                        "sync_info": {"on_update": [], "on_wait": [w]},
                    })
                inst["sync_info"] = {"on_update": ou, "on_wait": ow[:1]}
                out.append(inst)
                continue
            if inst.get("opcode") == "Drain" and (len(ow) > 1 or len(ou) > 1):
                for w in ow[1:]:
                    counter[0] += 1
                    out.append({
                        "debug": inst.get("debug", 0),
                        "engine": inst["engine"],
                        "ins": [], "outs": [],
                        "name": f"synthwait-{counter[0]}",
                        "opcode": "EventSemaphore",
                        "sync_info": {"on_update": [], "on_wait": [w]},
                    })
                inst["sync_info"] = {"on_update": ou[:1], "on_wait": ow[:1]}
                out.append(inst)
                for u in ou[1:]:
                    counter[0] += 1
                    out.append({
                        "debug": inst.get("debug", 0),
                        "engine": inst["engine"],
                        "ins": [], "outs": [],
                        "name": f"synthupd-{counter[0]}",
                        "opcode": "EventSemaphore",
                        "sync_info": {"on_update": [u], "on_wait": []},
                    })
            else:
                out.append(inst)
        b["instructions"] = out
        for sb in b.get("blocks", []):
            fix_block(sb)

    for fn in d.get("functions", []):
        for blk in fn.get("blocks", []):
            fix_block(blk)
    return json.dumps(d).encode()


def _install_drainfix():
    import concourse.bass as bass
    if getattr(bass.Bass, "_drainfix_installed", False):
        return
    orig = bass.Bass.to_json_bytes

    def patched(self):
        return _fix_module_json(orig(self))

    bass.Bass.to_json_bytes = patched
    bass.Bass._drainfix_installed = True


# ---------------------------------------------------------------- program
def _build_nc(reps=1):
    import concourse.bass as bass
    import concourse.mybir as mybir
    from concourse.tile import TileContext
    from contextlib import ExitStack

    f32 = mybir.dt.float32
    f32r = mybir.dt.float32r
    bf16 = mybir.dt.bfloat16
    i16 = mybir.dt.int16
    EXP = mybir.ActivationFunctionType.Exp
    MUL = mybir.AluOpType.mult
    ADD = mybir.AluOpType.add

    nc = bass.Bass("TRN2", num_devices=NCORES)

    xqT = nc.dram_tensor("xqT", [D, S], f32, kind="ExternalInput")
    xkT = nc.dram_tensor("xkT", [D, S], f32, kind="ExternalInput")
    xvT = nc.dram_tensor("xvT", [D, S], f32, kind="ExternalInput")
    wq = nc.dram_tensor("wq", [D, CPC], f32, kind="ExternalInput")
    wk = nc.dram_tensor("wk", [D, CPC], f32, kind="ExternalInput")
    wv = nc.dram_tensor("wv", [D, CPC], f32, kind="ExternalInput")
    bqd = nc.dram_tensor("bq", [CPC], f32, kind="ExternalInput")
    bkd = nc.dram_tensor("bk", [CPC], f32, kind="ExternalInput")
    bvd = nc.dram_tensor("bv", [CPC], f32, kind="ExternalInput")
    outd = nc.dram_tensor("out", [CPC, S], f32, kind="ExternalOutput")
    # scratch for the denominator-reciprocal broadcast bounce (stride-0
    # partition reads are only legal from DRAM)
    recscr = nc.dram_tensor("recscr", [2 * NPH, SQC], f32, kind="Internal")

    with ExitStack() as ctx:
        ctx.enter_context(nc.allow_low_precision(
            reason="qk in bf16 + bit-trick exp; matmul accumulates f32; "
                   "rel-err gate 2e-2"))
        tc = ctx.enter_context(TileContext(nc))
        sb = ctx.enter_context(tc.tile_pool(name="sb", bufs=1))
        ps = ctx.enter_context(tc.tile_pool(name="ps", bufs=1, space="PSUM"))

        # ---- resident weights / constants (wk + xk first: critical) ----
        wk_sb = sb.tile([128, NDT, CPC], f32r, name="wk_sb")
        for dd in range(NDT):
            nc.sync.dma_start(out=wk_sb[:, dd, :],
                              in_=wk[dd * 128:(dd + 1) * 128, :].bitcast(f32r))
        bk_sb = sb.tile([128, NP], f32, name="bk_sb")
        nc.sync.dma_start(out=bk_sb, in_=bkd.rearrange("(p c) -> c p", p=NP))
        bq_sb = sb.tile([128, NP], f32, name="bq_sb")
        nc.sync.dma_start(out=bq_sb, in_=bqd.rearrange("(p c) -> c p", p=NP))

        # persistent activation tiles
        qT = [sb.tile([128, S], bf16, name=f"qT{p}") for p in range(NP)]
        kT = [sb.tile([128, S], bf16, name=f"kT{p}") for p in range(NP)]
        v_aug = sb.tile([128, NSK, 8, 65], bf16, name="v_aug")

        # ---- helpers ----
        def load_x(x_dram, t):
            xts = []
            for dd in range(NDT):
                x_t = sb.tile([128, SQC], f32r, tag="xs", bufs=12,
                              name=f"x_{t}_{dd}")
                nc.sync.dma_start(
                    out=x_t,
                    in_=x_dram[dd * 128:(dd + 1) * 128,
                               t * SQC:(t + 1) * SQC].bitcast(f32r))
                xts.append(x_t)
            return xts

        def emit_proj(t, plist, w_sb, x_dram, bias_sb, dst):
            """dst[p][:, t*512:(t+1)*512] = (x @ W + b).T chunk (bf16)."""
            xts = load_x(x_dram, t)
            for p in plist:
                pr = ps.tile([128, SQC], f32, tag="vp", bufs=2,
                             name=f"prj_{t}_{p}")
                for dd in range(NDT):
                    nc.tensor.matmul(
                        pr[:, :],
                        w_sb[:, dd, p * 128:(p + 1) * 128],
                        xts[dd][:, :],
                        start=(dd == 0), stop=(dd == NDT - 1))
                nc.vector.tensor_scalar_add(
                    dst[p][:, t * SQC:(t + 1) * SQC], pr[:, :],
                    bias_sb[:, p:p + 1])

        def emit_proj_v(tt):
            """v_aug[:, tt, h, 0:64] = (xv @ Wv + bv) rows tt*128.., bf16."""
            xvt = []
            for dd in range(NDT):
                xv_t = sb.tile([128, 128], f32r, tag="xv", bufs=10,
                               name=f"xv_{tt}_{dd}")
                nc.sync.dma_start(
                    out=xv_t,
                    in_=xvT[dd * 128:(dd + 1) * 128,
                            tt * 128:(tt + 1) * 128].bitcast(f32r))
                xvt.append(xv_t)
            pv = ps.tile([128, CPC], f32, tag="vp", bufs=2, name=f"pv_{tt}")
            for dd in range(NDT):
                nc.tensor.matmul(
                    pv[:, :], xvt[dd][:, :], wv_sb[:, dd, :],
                    start=(dd == 0), stop=(dd == NDT - 1))
            nc.vector.tensor_add(
                v_aug[:, tt, :, 0:64],
                pv.rearrange("c (h d) -> c h d", h=8),
                bv_bc.rearrange("c (h d) -> c h d", h=8))

        alpha = {}     # (c, h, i) -> [128, 512] bf16
        av_tiles = {}  # c -> {h: psum tile [65, 512]}

        def emit_qk_i(c, i):
            p, j = divmod(c, NJ)
            for h in range(2):
                sc = ps.tile([128, SQC], f32, tag="sc", bufs=3,
                             name=f"sc_{c}_{i}_{h}")
                nc.tensor.matmul(
                    sc[:, :],
                    kT[p][h * 64:(h + 1) * 64, i * 128:(i + 1) * 128],
                    qT[p][h * 64:(h + 1) * 64, j * SQC:(j + 1) * SQC],
                    start=True, stop=True)
                a_t = sb.tile([128, SQC], bf16, tag="alpha", bufs=36,
                              name=f"al_{c}_{i}_{h}")
                m = (2 * i + h) % 16
                if m in DVE_SLOTS:
                    nc.vector.tensor_scalar(
                        a_t.bitcast(i16), sc[:, :], EXPA, EXPB, MUL, ADD)
                else:
                    nc.scalar.activation(a_t[:, :], sc[:, :], EXP, scale=0.125)
                alpha[(c, h, i)] = a_t

        def emit_av_i(c, i):
            p, j = divmod(c, NJ)
            if i == 0:
                av_tiles[c] = {
                    h: ps.tile([65, SQC], f32, tag="av", bufs=3,
                               name=f"av_{c}_{h}")
                    for h in range(2)}
            for h in range(2):
                a_t = alpha.pop((c, h, i))
                nc.tensor.matmul(
                    av_tiles[c][h][:, :],
                    v_aug[:, i, 2 * p + h, :],
                    a_t[:, :],
                    start=(i == 0), stop=(i == NSK - 1))

        def emit_norm(c):
            """Normalize + write out chunk c. No PE instructions: DVE
            reciprocal of the denominator row, DRAM-bounce broadcast to 64
            partitions, DVE multiply. av psum tiles are freed by the DVE
            reads."""
            p, j = divmod(c, NJ)
            avt = av_tiles.pop(c)
            for h in range(2):
                r0 = (2 * p + h) * 64
                slot = 2 * c + h
                rec = sb.tile([1, SQC], f32, tag="rec", bufs=4,
                              name=f"rec_{c}_{h}")
                nc.vector.reciprocal(rec[:, :], avt[h][64:65, :])
                nc.sync.dma_start(out=recscr[slot:slot + 1, :],
                                  in_=rec[:, :])
                recB = sb.tile([64, SQC], f32, tag="recB", bufs=4,
                               name=f"recB_{c}_{h}")
                _r = recscr[slot:slot + 1, :]
                nc.sync.dma_start(
                    out=recB,
                    in_=bass.AP(tensor=_r.tensor, offset=_r.offset,
                                ap=[[0, 64]] + list(_r.ap)[1:]))
                cx = sb.tile([64, SQC], f32, tag="cx", bufs=4,
                             name=f"cx_{c}_{h}")
                nc.vector.tensor_mul(cx[:, :], avt[h][0:64, :], recB[:, :])
                nc.sync.dma_start(
                    out=outd[r0:r0 + 64, j * SQC:(j + 1) * SQC],
                    in_=cx[:, :])

        # late-loaded residents (emitted after critical-path DMAs above,
        # but data only needed from mid-prologue onwards)
        wq_sb = sb.tile([128, NDT, CPC], f32r, name="wq_sb")
        wv_sb = sb.tile([128, NDT, CPC], f32r, name="wv_sb")
        bv_bc = sb.tile([128, CPC], f32, name="bv_bc")

        def _emit_late_consts():
            for dd in range(NDT):
                nc.sync.dma_start(
                    out=wq_sb[:, dd, :],
                    in_=wq[dd * 128:(dd + 1) * 128, :].bitcast(f32r))
            for dd in range(NDT):
                nc.sync.dma_start(
                    out=wv_sb[:, dd, :],
                    in_=wv[dd * 128:(dd + 1) * 128, :].bitcast(f32r))
            _bva = bvd[:]
            nc.sync.dma_start(
                out=bv_bc,
                in_=bass.AP(tensor=_bva.tensor, offset=_bva.offset,
                            ap=[[0, 128]] + list(_bva.ap)))
            nc.gpsimd.memset(v_aug[:, :, :, 64:65], 1.0)

        # ---- emission schedule ----
        def _emit_all():
            # prologue: kT all-p (phase 0 needs the full kT[0]), then qT t0,
            # then phase-0 QK interleaved with qT t1..3.
            for t in range(NJ):
                emit_proj(t, [0, 1, 2, 3], wk_sb, xkT, bk_sb, kT)
                if t == 0:
                    _emit_late_consts()
            emit_proj(0, [0, 1, 2, 3], wq_sb, xqT, bq_sb, qT)
            for i in range(4):
                emit_qk_i(0, i)
            emit_proj(1, [0, 1, 2, 3], wq_sb, xqT, bq_sb, qT)
            for i in range(4, 8):
                emit_qk_i(0, i)
            emit_proj(2, [0, 1, 2, 3], wq_sb, xqT, bq_sb, qT)
            for i in range(8, 12):
                emit_qk_i(0, i)
            emit_proj(3, [0, 1, 2, 3], wq_sb, xqT, bq_sb, qT)
            for i in range(12, 16):
                emit_qk_i(0, i)

            # phase 1: v-projection (xv DMA paced) interleaved with QK(1)
            # and AV(0) (AV(0,i) needs v_aug[:, i] just computed).
            for i in range(NSK):
                emit_proj_v(i)
                emit_qk_i(1, i)
                emit_av_i(0, i)
            emit_norm(0)

            # phases 2..15: pure QK/AV
            for c in range(2, NPH):
                for i in range(NSK):
                    emit_qk_i(c, i)
                    emit_av_i(c - 1, i)
                emit_norm(c - 1)

            # epilogue
            for i in range(NSK):
                emit_av_i(NPH - 1, i)
            emit_norm(NPH - 1)

        for _rep in range(reps):
            _emit_all()

    return nc


_NC_BY_REPS = {}


def _get_nc(reps=1):
    if reps not in _NC_BY_REPS:
        _install_drainfix()
        _NC_BY_REPS[reps] = _build_nc(reps)
    return _NC_BY_REPS[reps]


# ---------------------------------------------------------------- entry
def kernel(query, key_in, value, Wq, bq, Wk, bk, Wv, bv):
    from concourse.bass_utils import run_bass_kernel_spmd

    nc = _get_nc()
    query = np.asarray(query, np.float32)
    key_in = np.asarray(key_in, np.float32)
    value = np.asarray(value, np.float32)
    Wq = np.asarray(Wq, np.float32)
    Wk = np.asarray(Wk, np.float32)
    Wv = np.asarray(Wv, np.float32)
    bq = np.asarray(bq, np.float32)
    bk = np.asarray(bk, np.float32)
    bv = np.asarray(bv, np.float32)

    in_maps = []
    for c in range(NCORES):
        b, hg = divmod(c, 2)
        cols = slice(hg * CPC, (hg + 1) * CPC)
        in_maps.append({
            "xqT": np.ascontiguousarray(query[b].T),
            "xkT": np.ascontiguousarray(key_in[b].T),
            "xvT": np.ascontiguousarray(value[b].T),
            "wq": np.ascontiguousarray(Wq[:, cols]),
            "wk": np.ascontiguousarray(Wk[:, cols]),
            "wv": np.ascontiguousarray(Wv[:, cols]),
            "bq": np.ascontiguousarray(bq[cols]),
            "bk": np.ascontiguousarray(bk[cols]),
            "bv": np.ascontiguousarray(bv[cols]),
        })

    res = run_bass_kernel_spmd(nc, in_maps, core_ids=list(range(NCORES)))

    out = np.empty((B, S, D), np.float32)
    for c in range(NCORES):
        b, hg = divmod(c, 2)
        out[b, :, hg * CPC:(hg + 1) * CPC] = res.results[c]["out"].T
    return out


# revision 22
# speedup vs baseline: 1.6001x; 1.2658x over previous
"""nn_CrossAttention kernel v4 for 8x TRN2 NeuronCores.

Sharding: core c = (batch b = c//2, head-group hg = c%2 of 8 heads).

v4 design (v3 = 590us, v2 baseline = 746us):
 - v3 post-mortem: HAM oscillated 16x (41% of time at K=4/8). Root
   cause: at every phase boundary AV(c,0,h1) waited 3-5us for the norm
   chain (reciprocal 3.4us! -> DRAM bounce -> mul) to free an av PSUM
   bank -> MID re-throttle each phase. Also DVE ops pay a pipe-flush
   DRAIN, so 12 trick slots + norm saturated DVE.
 - Fixes:
   * Quick av->SBUF copies (ACT h0 / DVE h1, ~1us) free the av banks
     immediately at the boundary; norm runs off the copies.
   * reciprocal_approx_fast (~0.7us vs 3.4us, 51 ULP - plenty).
   * exp at 1024-wide over sc bank-PAIRS: [128,2,512] psum tiles, one
     exp op covers both heads of an i-slot. ACT 12 pairs + DVE 4 pairs
     (bit-trick exp, 25% of slots -> ~1e-2 total rel err).
   * Prologue: interleave kT t-passes with phase-0 QK blocks, qT t0
     right after wq; first QK at ~40us instead of 83us.
 - PSUM: vp 1 + sc 2x2 + av 3 = 8 banks.
"""

import json
import numpy as np

B, S, D, NH, HD = 4, 2048, 1024, 16, 64
CPC = 512          # cols per core = 8 heads * 64
NCORES = 8
NDT = D // 128     # 8 d-tiles
NP = CPC // 128    # 4 c-tiles (head pairs)
NSK = S // 128     # 16 sk-tiles
NJ = S // 512      # 4 sq chunks
SQC = 512          # sq chunk size
NPH = NP * NJ      # 16 phases

# bit-trick exp constants: bf16 bits = round(score * EXPA + EXPB)
# exp(s*0.125) = 2^(s*0.125*log2 e); bf16 bits = exp_field*128 + mantissa
EXPA = 0.125 * 1.4426950408889634 * 128.0   # 23.0831...
EXPB = 16256.0 - 7.5                        # 127*128 - sigma (sigma tuned)

# i-slot -> engine assignment: i % 4 == 2 -> DVE trick (4 of 16 pairs)
DVE_PAIRS = frozenset((2,))


# ---------------------------------------------------------------- drain fix
def _fix_module_json(bj: bytes) -> bytes:
    """This walrus build accepts at most ONE sync wait/update on CTRL-lowered
    instructions (Drain). Move extras onto EventSemaphore instructions."""
    d = json.loads(bj)
    counter = [0]

    def fix_block(b):
        out = []
        for inst in b.get("instructions", []):
            si = inst.get("sync_info") or {}
            ow = si.get("on_wait") or []
            ou = si.get("on_update") or []
            if (inst.get("opcode") not in
                    ("EventSemaphore", "Call", "RegisterMove",
                     "UnconditionalBranch", "ISA", "Drain") and len(ow) > 1):
                for w in ow[1:]:
                    counter[0] += 1
                    out.append({
                        "debug": inst.get("debug", 0),
                        "engine": inst["engine"],
                        "ins": [], "outs": [],
                        "name": f"synthmmw-{counter[0]}",
                        "opcode": "EventSemaphore",
                        "sync_info": {"on_update": [], "on_wait": [w]},
                    })
                inst["sync_info"] = {"on_update": ou, "on_wait": ow[:1]}
                out.append(inst)
                continue
            if inst.get("opcode") == "Drain" and (len(ow) > 1 or len(ou) > 1):
                for w in ow[1:]:
                    counter[0] += 1
                    out.append({
                        "debug": inst.get("debug", 0),
                        "engine": inst["engine"],
                        "ins": [], "outs": [],
                        "name": f"synthwait-{counter[0]}",
                        "opcode": "EventSemaphore",
                        "sync_info": {"on_update": [], "on_wait": [w]},
                    })
                inst["sync_info"] = {"on_update": ou[:1], "on_wait": ow[:1]}
                out.append(inst)
                for u in ou[1:]:
                    counter[0] += 1
                    out.append({
                        "debug": inst.get("debug", 0),
                        "engine": inst["engine"],
                        "ins": [], "outs": [],
                        "name": f"synthupd-{counter[0]}",
                        "opcode": "EventSemaphore",
                        "sync_info": {"on_update": [u], "on_wait": []},
                    })
            else:
                out.append(inst)
        b["instructions"] = out
        for sb in b.get("blocks", []):
            fix_block(sb)

    for fn in d.get("functions", []):
        for blk in fn.get("blocks", []):
            fix_block(blk)
    return json.dumps(d).encode()


def _install_drainfix():
    import concourse.bass as bass
    if getattr(bass.Bass, "_drainfix_installed", False):
        return
    orig = bass.Bass.to_json_bytes

    def patched(self):
        return _fix_module_json(orig(self))

    bass.Bass.to_json_bytes = patched
    bass.Bass._drainfix_installed = True


# ---------------------------------------------------------------- program
def _build_nc(reps=1):
    import concourse.bass as bass
    import concourse.mybir as mybir
    from concourse.tile import TileContext
    from contextlib import ExitStack

    f32 = mybir.dt.float32
    f32r = mybir.dt.float32r
    bf16 = mybir.dt.bfloat16
    i16 = mybir.dt.int16
    EXP = mybir.ActivationFunctionType.Exp
    MUL = mybir.AluOpType.mult
    ADD = mybir.AluOpType.add

    nc = bass.Bass("TRN2", num_devices=NCORES)

    xqT = nc.dram_tensor("xqT", [D, S], bf16, kind="ExternalInput")
    xkT = nc.dram_tensor("xkT", [D, S], bf16, kind="ExternalInput")
    xvT = nc.dram_tensor("xvT", [D, S], bf16, kind="ExternalInput")
    wq = nc.dram_tensor("wq", [D, CPC], bf16, kind="ExternalInput")
    wk = nc.dram_tensor("wk", [D, CPC], bf16, kind="ExternalInput")
    wv = nc.dram_tensor("wv", [D, CPC], bf16, kind="ExternalInput")
    bqd = nc.dram_tensor("bq", [CPC], f32, kind="ExternalInput")
    bkd = nc.dram_tensor("bk", [CPC], f32, kind="ExternalInput")
    bvd = nc.dram_tensor("bv", [CPC], f32, kind="ExternalInput")
    outd = nc.dram_tensor("out", [CPC, S], f32, kind="ExternalOutput")
    # scratch for the denominator-reciprocal broadcast bounce (stride-0
    # partition reads are only legal from DRAM)
    recscr = nc.dram_tensor("recscr", [2 * NPH, SQC], f32, kind="Internal")

    with ExitStack() as ctx:
        ctx.enter_context(nc.allow_low_precision(
            reason="qk in bf16 + bit-trick exp; matmul accumulates f32; "
                   "rel-err gate 2e-2"))
        tc = ctx.enter_context(TileContext(nc))
        sb = ctx.enter_context(tc.tile_pool(name="sb", bufs=1))
        ps = ctx.enter_context(tc.tile_pool(name="ps", bufs=1, space="PSUM"))

        # ---- resident weights / constants (wk + xk first: critical) ----
        wk_sb = sb.tile([128, NDT, CPC], bf16, name="wk_sb")
        for dd in range(NDT):
            nc.sync.dma_start(out=wk_sb[:, dd, :],
                              in_=wk[dd * 128:(dd + 1) * 128, :])
        bk_sb = sb.tile([128, NP], f32, name="bk_sb")
        nc.sync.dma_start(out=bk_sb, in_=bkd.rearrange("(p c) -> c p", p=NP))
        bq_sb = sb.tile([128, NP], f32, name="bq_sb")
        nc.sync.dma_start(out=bq_sb, in_=bqd.rearrange("(p c) -> c p", p=NP))

        # persistent activation tiles
        qT = [sb.tile([128, S], bf16, name=f"qT{p}") for p in range(NP)]
        kT = [sb.tile([128, S], bf16, name=f"kT{p}") for p in range(NP)]
        v_aug = sb.tile([128, NSK, 8, 65], bf16, name="v_aug")

        # ---- helpers ----
        def load_x2(x_dram, u):
            """8 [128, 1024] bf16 tiles covering t-chunks 2u, 2u+1 (2KB
            per-partition DMA lines)."""
            xts = []
            for dd in range(NDT):
                x_t = sb.tile([128, 2 * SQC], bf16, tag="xs", bufs=20,
                              name=f"x_{u}_{dd}")
                nc.sync.dma_start(
                    out=x_t,
                    in_=x_dram[dd * 128:(dd + 1) * 128,
                               u * 2 * SQC:(u + 1) * 2 * SQC])
                xts.append(x_t)
            return xts

        def emit_proj(t, plist, w_sb, bias_sb, dst, xts):
            """dst[p][:, t*512:(t+1)*512] = (x @ W + b).T chunk (bf16).
            xts are the [128, 1024] pair tiles for u = t//2."""
            xo = (t % 2) * SQC
            for p in plist:
                pr = ps.tile([128, SQC], f32, tag="vp", bufs=1,
                             name=f"prj_{t}_{p}")
                for dd in range(NDT):
                    nc.tensor.matmul(
                        pr[:, :],
                        w_sb[:, dd, p * 128:(p + 1) * 128],
                        xts[dd][:, xo:xo + SQC],
                        start=(dd == 0), stop=(dd == NDT - 1))
                nc.vector.tensor_scalar_add(
                    dst[p][:, t * SQC:(t + 1) * SQC], pr[:, :],
                    bias_sb[:, p:p + 1])

        xv_group = {}  # g -> list of 8 [128, 512] bf16 tiles (tt 4g..4g+3)

        def load_xv(g):
            xvt = []
            for dd in range(NDT):
                xv_t = sb.tile([128, 4 * 128], bf16, tag="xv", bufs=12,
                               name=f"xv_{g}_{dd}")
                nc.sync.dma_start(
                    out=xv_t,
                    in_=xvT[dd * 128:(dd + 1) * 128,
                            g * 512:(g + 1) * 512])
                xvt.append(xv_t)
            xv_group[g] = xvt

        def emit_proj_v(tt):
            """v_aug[:, tt, h, 0:64] = (xv @ Wv + bv) rows tt*128.., bf16."""
            xvt = xv_group[tt // 4]
            co = (tt % 4) * 128
            pv = ps.tile([128, CPC], f32, tag="vp", bufs=1, name=f"pv_{tt}")
            for dd in range(NDT):
                nc.tensor.matmul(
                    pv[:, :], xvt[dd][:, co:co + 128], wv_sb[:, dd, :],
                    start=(dd == 0), stop=(dd == NDT - 1))
            nc.vector.tensor_add(
                v_aug[:, tt, :, 0:64],
                pv.rearrange("c (h d) -> c h d", h=8),
                bv_bc.rearrange("c (h d) -> c h d", h=8))
            if tt % 4 == 3:
                xv_group.pop(tt // 4)

        alpha = {}     # (c, i) -> [128, 2, 512] bf16 pair tile
        av_tiles = {}  # c -> {h: psum tile [65, 512]}

        def emit_qk_i(c, i):
            p, j = divmod(c, NJ)
            scp = ps.tile([128, 2, SQC], f32, tag="sc", bufs=2,
                          name=f"sc_{c}_{i}")
            for h in range(2):
                nc.tensor.matmul(
                    scp[:, h, :],
                    kT[p][h * 64:(h + 1) * 64, i * 128:(i + 1) * 128],
                    qT[p][h * 64:(h + 1) * 64, j * SQC:(j + 1) * SQC],
                    start=True, stop=True)
            a_p = sb.tile([128, 2, SQC], bf16, tag="alpha", bufs=18,
                          name=f"al_{c}_{i}")
            if i % 4 in DVE_PAIRS:
                nc.vector.tensor_scalar(
                    a_p.bitcast(i16), scp[:, :, :], EXPA, EXPB, MUL, ADD)
            else:
                nc.scalar.activation(a_p[:, :, :], scp[:, :, :], EXP,
                                     scale=0.125)
            alpha[(c, i)] = a_p

        def emit_av_i(c, i):
            p, j = divmod(c, NJ)
            if i == 0:
                av_tiles[c] = {
                    h: ps.tile([65, SQC], f32, tag="av", bufs=3,
                               name=f"av_{c}_{h}")
                    for h in range(2)}
            a_p = alpha[(c, i)]
            for h in range(2):
                nc.tensor.matmul(
                    av_tiles[c][h][:, :],
                    v_aug[:, i, 2 * p + h, :],
                    a_p[:, h, :],
                    start=(i == 0), stop=(i == NSK - 1))
            alpha.pop((c, i))

        def emit_norm(c):
            """Free the av PSUM banks FAST (ACT copies h0, DVE copies h1 to
            SBUF ~1us), then normalize off the copies: approx-reciprocal of
            the denominator row, DRAM-bounce broadcast to 64 partitions,
            DVE multiply, DMA out."""
            p, j = divmod(c, NJ)
            avt = av_tiles.pop(c)
            avs = {}
            for h in range(2):
                avs[h] = sb.tile([65, SQC], f32, tag="avs", bufs=4,
                                 name=f"avs_{c}_{h}")
            nc.scalar.copy(avs[0][:, :], avt[0][:, :])
            nc.vector.tensor_copy(avs[1][:, :], avt[1][:, :])
            for h in range(2):
                r0 = (2 * p + h) * 64
                slot = 2 * c + h
                # bounce the raw denominator row out, read it back spread
                # over 64 partitions ([64,8]) so the 6-cycles/elem DVE
                # reciprocal costs ~8 elems/lane instead of 512.
                nc.sync.dma_start(out=recscr[slot:slot + 1, :],
                                  in_=avs[h][64:65, :])
                _r = recscr[slot:slot + 1, :]
                den8 = sb.tile([64, 8], f32, tag="den8", bufs=4,
                               name=f"den8_{c}_{h}")
                nc.sync.dma_start(
                    out=den8,
                    in_=bass.AP(tensor=_r.tensor, offset=_r.offset,
                                ap=[[8, 64], [1, 8]]))
                rec8 = sb.tile([64, 8], f32, tag="rec8", bufs=4,
                               name=f"rec8_{c}_{h}")
                nc.vector.reciprocal(rec8[:, :], den8[:, :])
                # bounce the reciprocal back and broadcast-read [64, 512]
                nc.sync.dma_start(
                    out=bass.AP(tensor=_r.tensor, offset=_r.offset,
                                ap=[[8, 64], [1, 8]]),
                    in_=rec8[:, :])
                recB = sb.tile([64, SQC], f32, tag="recB", bufs=4,
                               name=f"recB_{c}_{h}")
                nc.sync.dma_start(
                    out=recB,
                    in_=bass.AP(tensor=_r.tensor, offset=_r.offset,
                                ap=[[0, 64]] + list(_r.ap)[1:]))
                cx = sb.tile([64, SQC], f32, tag="cx", bufs=4,
                             name=f"cx_{c}_{h}")
                nc.vector.tensor_mul(cx[:, :], avs[h][0:64, :], recB[:, :])
                nc.sync.dma_start(
                    out=outd[r0:r0 + 64, j * SQC:(j + 1) * SQC],
                    in_=cx[:, :])

        # late-loaded residents (emitted after critical-path DMAs above,
        # but data only needed from mid-prologue onwards)
        wq_sb = sb.tile([128, NDT, CPC], bf16, name="wq_sb")
        wv_sb = sb.tile([128, NDT, CPC], bf16, name="wv_sb")
        bv_bc = sb.tile([128, CPC], f32, name="bv_bc")

        def _emit_wq():
            for dd in range(NDT):
                nc.sync.dma_start(
                    out=wq_sb[:, dd, :],
                    in_=wq[dd * 128:(dd + 1) * 128, :])

        def _emit_wv():
            for dd in range(NDT):
                nc.sync.dma_start(
                    out=wv_sb[:, dd, :],
                    in_=wv[dd * 128:(dd + 1) * 128, :])
            _bva = bvd[:]
            nc.sync.dma_start(
                out=bv_bc,
                in_=bass.AP(tensor=_bva.tensor, offset=_bva.offset,
                            ap=[[0, 128]] + list(_bva.ap)))
            nc.gpsimd.memset(v_aug[:, :, :, 64:65], 1.0)

        # ---- emission schedule ----
        def _emit_all():
            # prologue (x/W in bf16, [128,1024] x pair-tiles): kT t0/t1 ->
            # qT t0/t1 -> phase-0 QK 0..7 -> kT t2/t3 -> QK 8..15 ->
            # qT t2/t3 -> wv. wq DMA right after xk u0 so the first qT
            # chain isn't blocked.
            xk0 = load_x2(xkT, 0)
            _emit_wq()
            emit_proj(0, [0, 1, 2, 3], wk_sb, bk_sb, kT, xk0)
            emit_proj(1, [0, 1, 2, 3], wk_sb, bk_sb, kT, xk0)
            xq0 = load_x2(xqT, 0)
            emit_proj(0, [0, 1, 2, 3], wq_sb, bq_sb, qT, xq0)
            for i in range(4):
                emit_qk_i(0, i)
            xk1 = load_x2(xkT, 1)
            emit_proj(2, [0, 1, 2, 3], wk_sb, bk_sb, kT, xk1)
            for i in range(4, 8):
                emit_qk_i(0, i)
            emit_proj(3, [0, 1, 2, 3], wk_sb, bk_sb, kT, xk1)
            for i in range(8, 12):
                emit_qk_i(0, i)
            emit_proj(1, [0, 1, 2, 3], wq_sb, bq_sb, qT, xq0)
            for i in range(12, 16):
                emit_qk_i(0, i)
            xq1 = load_x2(xqT, 1)
            emit_proj(2, [0, 1, 2, 3], wq_sb, bq_sb, qT, xq1)
            emit_proj(3, [0, 1, 2, 3], wq_sb, bq_sb, qT, xq1)
            _emit_wv()

            # phase 1: v-projection (xv DMA paced) interleaved with QK(1)
            # and AV(0) (AV(0,i) needs v_aug[:, i] just computed).
            for i in range(NSK):
                if i == 0:
                    load_xv(0)
                if i % 4 == 1 and i // 4 + 1 < 4:
                    load_xv(i // 4 + 1)
                emit_proj_v(i)
                emit_qk_i(1, i)
                emit_av_i(0, i)
            emit_norm(0)

            # phases 2..15: pure QK/AV
            for c in range(2, NPH):
                for i in range(NSK):
                    emit_qk_i(c, i)
                    emit_av_i(c - 1, i)
                emit_norm(c - 1)

            # epilogue
            for i in range(NSK):
                emit_av_i(NPH - 1, i)
            emit_norm(NPH - 1)

        for _rep in range(reps):
            _emit_all()

    return nc


_NC_BY_REPS = {}


def _get_nc(reps=1):
    if reps not in _NC_BY_REPS:
        _install_drainfix()
        _NC_BY_REPS[reps] = _build_nc(reps)
    return _NC_BY_REPS[reps]


# ---------------------------------------------------------------- entry
def build_in_maps(inputs):
    import ml_dtypes

    bf16 = ml_dtypes.bfloat16
    query = np.asarray(inputs["query"], np.float32)
    key_in = np.asarray(inputs["key_in"], np.float32)
    value = np.asarray(inputs["value"], np.float32)
    Wq = np.asarray(inputs["Wq"], np.float32)
    Wk = np.asarray(inputs["Wk"], np.float32)
    Wv = np.asarray(inputs["Wv"], np.float32)
    bq = np.asarray(inputs["bq"], np.float32)
    bk = np.asarray(inputs["bk"], np.float32)
    bv = np.asarray(inputs["bv"], np.float32)

    in_maps = []
    for c in range(NCORES):
        b, hg = divmod(c, 2)
        cols = slice(hg * CPC, (hg + 1) * CPC)
        in_maps.append({
            "xqT": np.ascontiguousarray(query[b].T.astype(bf16)),
            "xkT": np.ascontiguousarray(key_in[b].T.astype(bf16)),
            "xvT": np.ascontiguousarray(value[b].T.astype(bf16)),
            "wq": np.ascontiguousarray(Wq[:, cols].astype(bf16)),
            "wk": np.ascontiguousarray(Wk[:, cols].astype(bf16)),
            "wv": np.ascontiguousarray(Wv[:, cols].astype(bf16)),
            "bq": np.ascontiguousarray(bq[cols]),
            "bk": np.ascontiguousarray(bk[cols]),
            "bv": np.ascontiguousarray(bv[cols]),
        })
    return in_maps


def kernel(query, key_in, value, Wq, bq, Wk, bk, Wv, bv):
    from concourse.bass_utils import run_bass_kernel_spmd

    nc = _get_nc()
    in_maps = build_in_maps({
        "query": query, "key_in": key_in, "value": value,
        "Wq": Wq, "bq": bq, "Wk": Wk, "bk": bk, "Wv": Wv, "bv": bv,
    })

    res = run_bass_kernel_spmd(nc, in_maps, core_ids=list(range(NCORES)))

    out = np.empty((B, S, D), np.float32)
    for c in range(NCORES):
        b, hg = divmod(c, 2)
        out[b, :, hg * CPC:(hg + 1) * CPC] = res.results[c]["out"].T
    return out


# revision 28
# speedup vs baseline: 1.9110x; 1.1943x over previous
"""nn_CrossAttention kernel v4 for 8x TRN2 NeuronCores.

Sharding: core c = (batch b = c//2, head-group hg = c%2 of 8 heads).

v4 design (v3 = 590us, v2 baseline = 746us):
 - v3 post-mortem: HAM oscillated 16x (41% of time at K=4/8). Root
   cause: at every phase boundary AV(c,0,h1) waited 3-5us for the norm
   chain (reciprocal 3.4us! -> DRAM bounce -> mul) to free an av PSUM
   bank -> MID re-throttle each phase. Also DVE ops pay a pipe-flush
   DRAIN, so 12 trick slots + norm saturated DVE.
 - Fixes:
   * Quick av->SBUF copies (ACT h0 / DVE h1, ~1us) free the av banks
     immediately at the boundary; norm runs off the copies.
   * reciprocal_approx_fast (~0.7us vs 3.4us, 51 ULP - plenty).
   * exp at 1024-wide over sc bank-PAIRS: [128,2,512] psum tiles, one
     exp op covers both heads of an i-slot. ACT 12 pairs + DVE 4 pairs
     (bit-trick exp, 25% of slots -> ~1e-2 total rel err).
   * Prologue: interleave kT t-passes with phase-0 QK blocks, qT t0
     right after wq; first QK at ~40us instead of 83us.
 - PSUM: vp 1 + sc 2x2 + av 3 = 8 banks.
"""

import json
import numpy as np

B, S, D, NH, HD = 4, 2048, 1024, 16, 64
CPC = 512          # cols per core = 8 heads * 64
NCORES = 8
NDT = D // 128     # 8 d-tiles
NP = CPC // 128    # 4 c-tiles (head pairs)
NSK = S // 128     # 16 sk-tiles
NJ = S // 512      # 4 sq chunks
SQC = 512          # sq chunk size
NPH = NP * NJ      # 16 phases

# bit-trick exp constants: bf16 bits = round(score * EXPA + EXPB)
# exp(s*0.125) = 2^(s*0.125*log2 e); bf16 bits = exp_field*128 + mantissa
EXPA = 0.125 * 1.4426950408889634 * 128.0   # 23.0831...
EXPB = 16256.0 - 7.5                        # 127*128 - sigma (sigma tuned)

# i-slot -> engine assignment: i % 4 == 2 -> DVE trick (4 of 16 pairs)
DVE_PAIRS = frozenset((2,))


# ---------------------------------------------------------------- drain fix
def _fix_module_json(bj: bytes) -> bytes:
    """This walrus build accepts at most ONE sync wait/update on CTRL-lowered
    instructions (Drain). Move extras onto EventSemaphore instructions."""
    d = json.loads(bj)
    counter = [0]

    def fix_block(b):
        out = []
        for inst in b.get("instructions", []):
            si = inst.get("sync_info") or {}
            ow = si.get("on_wait") or []
            ou = si.get("on_update") or []
            if (inst.get("opcode") not in
                    ("EventSemaphore", "Call", "RegisterMove",
                     "UnconditionalBranch", "ISA", "Drain") and len(ow) > 1):
                for w in ow[1:]:
                    counter[0] += 1
                    out.append({
                        "debug": inst.get("debug", 0),
                        "engine": inst["engine"],
                        "ins": [], "outs": [],
                        "name": f"synthmmw-{counter[0]}",
                        "opcode": "EventSemaphore",
                        "sync_info": {"on_update": [], "on_wait": [w]},
                    })
                inst["sync_info"] = {"on_update": ou, "on_wait": ow[:1]}
                out.append(inst)
                continue
            if inst.get("opcode") == "Drain" and (len(ow) > 1 or len(ou) > 1):
                for w in ow[1:]:
                    counter[0] += 1
                    out.append({
                        "debug": inst.get("debug", 0),
                        "engine": inst["engine"],
                        "ins": [], "outs": [],
                        "name": f"synthwait-{counter[0]}",
                        "opcode": "EventSemaphore",
                        "sync_info": {"on_update": [], "on_wait": [w]},
                    })
                inst["sync_info"] = {"on_update": ou[:1], "on_wait": ow[:1]}
                out.append(inst)
                for u in ou[1:]:
                    counter[0] += 1
                    out.append({
                        "debug": inst.get("debug", 0),
                        "engine": inst["engine"],
                        "ins": [], "outs": [],
                        "name": f"synthupd-{counter[0]}",
                        "opcode": "EventSemaphore",
                        "sync_info": {"on_update": [u], "on_wait": []},
                    })
            else:
                out.append(inst)
        b["instructions"] = out
        for sb in b.get("blocks", []):
            fix_block(sb)

    for fn in d.get("functions", []):
        for blk in fn.get("blocks", []):
            fix_block(blk)
    return json.dumps(d).encode()


def _install_drainfix():
    import concourse.bass as bass
    if getattr(bass.Bass, "_drainfix_installed", False):
        return
    orig = bass.Bass.to_json_bytes

    def patched(self):
        return _fix_module_json(orig(self))

    bass.Bass.to_json_bytes = patched
    bass.Bass._drainfix_installed = True


# ---------------------------------------------------------------- program
def _build_nc(reps=1):
    import concourse.bass as bass
    import concourse.mybir as mybir
    from concourse.tile import TileContext
    from contextlib import ExitStack

    f32 = mybir.dt.float32
    f32r = mybir.dt.float32r
    bf16 = mybir.dt.bfloat16
    i16 = mybir.dt.int16
    EXP = mybir.ActivationFunctionType.Exp
    MUL = mybir.AluOpType.mult
    ADD = mybir.AluOpType.add

    nc = bass.Bass("TRN2", num_devices=NCORES)

    xqT = nc.dram_tensor("xqT", [D, S], bf16, kind="ExternalInput")
    xkT = nc.dram_tensor("xkT", [D, S], bf16, kind="ExternalInput")
    xvT = nc.dram_tensor("xvT", [D, S], bf16, kind="ExternalInput")
    wq = nc.dram_tensor("wq", [D, CPC], bf16, kind="ExternalInput")
    wk = nc.dram_tensor("wk", [D, CPC], bf16, kind="ExternalInput")
    wv = nc.dram_tensor("wv", [D, CPC], bf16, kind="ExternalInput")
    bqd = nc.dram_tensor("bq", [CPC], f32, kind="ExternalInput")
    bkd = nc.dram_tensor("bk", [CPC], f32, kind="ExternalInput")
    bvd = nc.dram_tensor("bv", [CPC], f32, kind="ExternalInput")
    outd = nc.dram_tensor("out", [CPC, S], f32, kind="ExternalOutput")
    # scratch for the denominator-reciprocal broadcast bounce (stride-0
    # partition reads are only legal from DRAM)
    recscr = nc.dram_tensor("recscr", [2 * NPH, SQC], f32, kind="Internal")

    with ExitStack() as ctx:
        ctx.enter_context(nc.allow_low_precision(
            reason="qk in bf16 + bit-trick exp; matmul accumulates f32; "
                   "rel-err gate 2e-2"))
        tc = ctx.enter_context(TileContext(nc))
        sb = ctx.enter_context(tc.tile_pool(name="sb", bufs=1))
        ps = ctx.enter_context(tc.tile_pool(name="ps", bufs=1, space="PSUM"))

        # ---- resident weights / constants (wk + xk first: critical) ----
        wk_sb = sb.tile([128, NDT, CPC], bf16, name="wk_sb")
        for dd in range(NDT):
            nc.sync.dma_start(out=wk_sb[:, dd, :],
                              in_=wk[dd * 128:(dd + 1) * 128, :])
        bk_sb = sb.tile([128, NP], f32, name="bk_sb")
        nc.sync.dma_start(out=bk_sb, in_=bkd.rearrange("(p c) -> c p", p=NP))
        bq_sb = sb.tile([128, NP], f32, name="bq_sb")
        nc.sync.dma_start(out=bq_sb, in_=bqd.rearrange("(p c) -> c p", p=NP))

        # persistent activation tiles
        qT = [sb.tile([128, S], bf16, name=f"qT{p}") for p in range(NP)]
        kT = [sb.tile([128, S], bf16, name=f"kT{p}") for p in range(NP)]
        v_aug = sb.tile([128, NSK, 8, 65], bf16, name="v_aug")

        # ---- helpers ----
        def load_x2(x_dram, u):
            """8 [128, 1024] bf16 tiles covering t-chunks 2u, 2u+1 (2KB
            per-partition DMA lines)."""
            xts = []
            for dd in range(NDT):
                x_t = sb.tile([128, 2 * SQC], bf16, tag="xs", bufs=12,
                              name=f"x_{u}_{dd}")
                nc.sync.dma_start(
                    out=x_t,
                    in_=x_dram[dd * 128:(dd + 1) * 128,
                               u * 2 * SQC:(u + 1) * 2 * SQC])
                xts.append(x_t)
            return xts

        def load_x1(x_dram, t):
            """8 single-t [128, 512] bf16 tiles (1KB lines; used for the qT
            passes so the critical path to the first QK is shorter)."""
            xts = []
            for dd in range(NDT):
                x_t = sb.tile([128, SQC], bf16, tag="xs1", bufs=12,
                              name=f"x1_{t}_{dd}")
                nc.sync.dma_start(
                    out=x_t,
                    in_=x_dram[dd * 128:(dd + 1) * 128,
                               t * SQC:(t + 1) * SQC])
                xts.append(x_t)
            return xts

        def emit_proj(t, plist, w_sb, bias_sb, dst, xts, single=False):
            """dst[p][:, t*512:(t+1)*512] = (x @ W + b).T chunk (bf16).
            xts are the [128, 1024] pair tiles for u = t//2 (or single-t
            [128, 512] tiles if single)."""
            xo = 0 if single else (t % 2) * SQC
            for p in plist:
                pr = ps.tile([128, SQC], f32, tag="vp", bufs=2,
                             name=f"prj_{t}_{p}")
                for dd in range(NDT):
                    nc.tensor.matmul(
                        pr[:, :],
                        w_sb[:, dd, p * 128:(p + 1) * 128],
                        xts[dd][:, xo:xo + SQC],
                        start=(dd == 0), stop=(dd == NDT - 1))
                nc.vector.tensor_scalar_add(
                    dst[p][:, t * SQC:(t + 1) * SQC], pr[:, :],
                    bias_sb[:, p:p + 1])

        xv_group = {}  # g -> list of 8 [128, 512] bf16 tiles (tt 4g..4g+3)

        def load_xv(g):
            xvt = []
            for dd in range(NDT):
                xv_t = sb.tile([128, 4 * 128], bf16, tag="xv", bufs=12,
                               name=f"xv_{g}_{dd}")
                nc.sync.dma_start(
                    out=xv_t,
                    in_=xvT[dd * 128:(dd + 1) * 128,
                            g * 512:(g + 1) * 512])
                xvt.append(xv_t)
            xv_group[g] = xvt

        def emit_proj_v(tt):
            """v_aug[:, tt, h, 0:64] = (xv @ Wv + bv) rows tt*128.., bf16."""
            xvt = xv_group[tt // 4]
            co = (tt % 4) * 128
            pv = ps.tile([128, CPC], f32, tag="vp", bufs=2, name=f"pv_{tt}")
            for dd in range(NDT):
                nc.tensor.matmul(
                    pv[:, :], xvt[dd][:, co:co + 128], wv_sb[:, dd, :],
                    start=(dd == 0), stop=(dd == NDT - 1))
            nc.vector.tensor_add(
                v_aug[:, tt, :, 0:64],
                pv.rearrange("c (h d) -> c h d", h=8),
                bv_bc.rearrange("c (h d) -> c h d", h=8))
            if tt % 4 == 3:
                xv_group.pop(tt // 4)

        alpha = {}     # (c, i) -> [128, 2, 512] bf16 pair tile
        av_tiles = {}  # c -> {h: psum tile [65, 512]}

        def emit_qk_i(c, i):
            p, j = divmod(c, NJ)
            scp = ps.tile([128, 2, SQC], f32, tag="sc", bufs=2,
                          name=f"sc_{c}_{i}")
            for h in range(2):
                nc.tensor.matmul(
                    scp[:, h, :],
                    kT[p][h * 64:(h + 1) * 64, i * 128:(i + 1) * 128],
                    qT[p][h * 64:(h + 1) * 64, j * SQC:(j + 1) * SQC],
                    start=True, stop=True)
            a_p = sb.tile([128, 2, SQC], bf16, tag="alpha", bufs=18,
                          name=f"al_{c}_{i}")
            if i % 4 in DVE_PAIRS:
                nc.vector.tensor_scalar(
                    a_p.bitcast(i16), scp[:, :, :], EXPA, EXPB, MUL, ADD)
            else:
                nc.scalar.activation(a_p[:, :, :], scp[:, :, :], EXP,
                                     scale=0.125)
            alpha[(c, i)] = a_p

        def emit_av_i(c, i):
            p, j = divmod(c, NJ)
            if i == 0:
                av_tiles[c] = {
                    h: ps.tile([65, SQC], f32, tag="av", bufs=2,
                               name=f"av_{c}_{h}")
                    for h in range(2)}
            a_p = alpha[(c, i)]
            for h in range(2):
                nc.tensor.matmul(
                    av_tiles[c][h][:, :],
                    v_aug[:, i, 2 * p + h, :],
                    a_p[:, h, :],
                    start=(i == 0), stop=(i == NSK - 1))
            alpha.pop((c, i))

        def emit_norm(c):
            """Free the av PSUM banks FAST (ACT copies h0, DVE copies h1 to
            SBUF ~1us), then normalize off the copies: approx-reciprocal of
            the denominator row, DRAM-bounce broadcast to 64 partitions,
            DVE multiply, DMA out."""
            p, j = divmod(c, NJ)
            avt = av_tiles.pop(c)
            avs = {}
            for h in range(2):
                avs[h] = sb.tile([65, SQC], f32, tag="avs", bufs=4,
                                 name=f"avs_{c}_{h}")
            nc.scalar.copy(avs[0][:, :], avt[0][:, :])
            nc.vector.tensor_copy(avs[1][:, :], avt[1][:, :])
            for h in range(2):
                r0 = (2 * p + h) * 64
                slot = 2 * c + h
                # bounce the raw denominator row out, read it back spread
                # over 64 partitions ([64,8]) so the 6-cycles/elem DVE
                # reciprocal costs ~8 elems/lane instead of 512.
                nc.sync.dma_start(out=recscr[slot:slot + 1, :],
                                  in_=avs[h][64:65, :])
                _r = recscr[slot:slot + 1, :]
                den8 = sb.tile([64, 8], f32, tag="den8", bufs=4,
                               name=f"den8_{c}_{h}")
                nc.sync.dma_start(
                    out=den8,
                    in_=bass.AP(tensor=_r.tensor, offset=_r.offset,
                                ap=[[8, 64], [1, 8]]))
                rec8 = sb.tile([64, 8], f32, tag="rec8", bufs=4,
                               name=f"rec8_{c}_{h}")
                nc.vector.reciprocal(rec8[:, :], den8[:, :])
                # bounce the reciprocal back and broadcast-read [64, 512]
                nc.sync.dma_start(
                    out=bass.AP(tensor=_r.tensor, offset=_r.offset,
                                ap=[[8, 64], [1, 8]]),
                    in_=rec8[:, :])
                recB = sb.tile([64, SQC], f32, tag="recB", bufs=4,
                               name=f"recB_{c}_{h}")
                nc.sync.dma_start(
                    out=recB,
                    in_=bass.AP(tensor=_r.tensor, offset=_r.offset,
                                ap=[[0, 64]] + list(_r.ap)[1:]))
                cx = sb.tile([64, SQC], f32, tag="cx", bufs=4,
                             name=f"cx_{c}_{h}")
                nc.vector.tensor_mul(cx[:, :], avs[h][0:64, :], recB[:, :])
                nc.sync.dma_start(
                    out=outd[r0:r0 + 64, j * SQC:(j + 1) * SQC],
                    in_=cx[:, :])

        # late-loaded residents (emitted after critical-path DMAs above,
        # but data only needed from mid-prologue onwards)
        wq_sb = sb.tile([128, NDT, CPC], bf16, name="wq_sb")
        wv_sb = sb.tile([128, NDT, CPC], bf16, name="wv_sb")
        bv_bc = sb.tile([128, CPC], f32, name="bv_bc")

        def _emit_wq():
            for dd in range(NDT):
                nc.sync.dma_start(
                    out=wq_sb[:, dd, :],
                    in_=wq[dd * 128:(dd + 1) * 128, :])

        def _emit_wv():
            for dd in range(NDT):
                nc.sync.dma_start(
                    out=wv_sb[:, dd, :],
                    in_=wv[dd * 128:(dd + 1) * 128, :])
            _bva = bvd[:]
            nc.sync.dma_start(
                out=bv_bc,
                in_=bass.AP(tensor=_bva.tensor, offset=_bva.offset,
                            ap=[[0, 128]] + list(_bva.ap)))
            nc.gpsimd.memset(v_aug[:, :, :, 64:65], 1.0)

        # ---- emission schedule ----
        def _emit_all():
            # prologue (x/W in bf16, [128,1024] x pair-tiles): kT t0/t1 ->
            # qT t0/t1 -> phase-0 QK 0..7 -> kT t2/t3 -> QK 8..15 ->
            # qT t2/t3 -> wv. wq DMA right after xk u0 so the first qT
            # chain isn't blocked.
            xk0 = load_x2(xkT, 0)
            _emit_wq()
            emit_proj(0, [0, 1, 2, 3], wk_sb, bk_sb, kT, xk0)
            emit_proj(1, [0, 1, 2, 3], wk_sb, bk_sb, kT, xk0)
            xq0 = load_x1(xqT, 0)
            emit_proj(0, [0, 1, 2, 3], wq_sb, bq_sb, qT, xq0, single=True)
            for i in range(4):
                emit_qk_i(0, i)
            xk1 = load_x2(xkT, 1)
            emit_proj(2, [0, 1, 2, 3], wk_sb, bk_sb, kT, xk1)
            for i in range(4, 8):
                emit_qk_i(0, i)
            emit_proj(3, [0, 1, 2, 3], wk_sb, bk_sb, kT, xk1)
            for i in range(8, 12):
                emit_qk_i(0, i)
            xq1 = load_x1(xqT, 1)
            emit_proj(1, [0, 1, 2, 3], wq_sb, bq_sb, qT, xq1, single=True)
            for i in range(12, 16):
                emit_qk_i(0, i)
            xq2 = load_x1(xqT, 2)
            emit_proj(2, [0, 1, 2, 3], wq_sb, bq_sb, qT, xq2, single=True)
            xq3 = load_x1(xqT, 3)
            emit_proj(3, [0, 1, 2, 3], wq_sb, bq_sb, qT, xq3, single=True)
            _emit_wv()

            # phase 1: v-projection (xv DMA paced) interleaved with QK(1)
            # and AV(0) (AV(0,i) needs v_aug[:, i] just computed).
            for i in range(NSK):
                if i == 0:
                    load_xv(0)
                if i % 4 == 1 and i // 4 + 1 < 4:
                    load_xv(i // 4 + 1)
                emit_proj_v(i)
                emit_qk_i(1, i)
                emit_av_i(0, i)
            emit_norm(0)

            # phases 2..15: pure QK/AV
            for c in range(2, NPH):
                for i in range(NSK):
                    emit_qk_i(c, i)
                    emit_av_i(c - 1, i)
                emit_norm(c - 1)

            # epilogue
            for i in range(NSK):
                emit_av_i(NPH - 1, i)
            emit_norm(NPH - 1)

        for _rep in range(reps):
            _emit_all()

    return nc


_NC_BY_REPS = {}


def _get_nc(reps=1):
    if reps not in _NC_BY_REPS:
        _install_drainfix()
        _NC_BY_REPS[reps] = _build_nc(reps)
    return _NC_BY_REPS[reps]


# ---------------------------------------------------------------- entry
def build_in_maps(inputs):
    import ml_dtypes

    bf16 = ml_dtypes.bfloat16
    query = np.asarray(inputs["query"], np.float32)
    key_in = np.asarray(inputs["key_in"], np.float32)
    value = np.asarray(inputs["value"], np.float32)
    Wq = np.asarray(inputs["Wq"], np.float32)
    Wk = np.asarray(inputs["Wk"], np.float32)
    Wv = np.asarray(inputs["Wv"], np.float32)
    bq = np.asarray(inputs["bq"], np.float32)
    bk = np.asarray(inputs["bk"], np.float32)
    bv = np.asarray(inputs["bv"], np.float32)

    in_maps = []
    for c in range(NCORES):
        b, hg = divmod(c, 2)
        cols = slice(hg * CPC, (hg + 1) * CPC)
        in_maps.append({
            "xqT": np.ascontiguousarray(query[b].T.astype(bf16)),
            "xkT": np.ascontiguousarray(key_in[b].T.astype(bf16)),
            "xvT": np.ascontiguousarray(value[b].T.astype(bf16)),
            "wq": np.ascontiguousarray(Wq[:, cols].astype(bf16)),
            "wk": np.ascontiguousarray(Wk[:, cols].astype(bf16)),
            "wv": np.ascontiguousarray(Wv[:, cols].astype(bf16)),
            "bq": np.ascontiguousarray(bq[cols]),
            "bk": np.ascontiguousarray(bk[cols]),
            "bv": np.ascontiguousarray(bv[cols]),
        })
    return in_maps


def kernel(query, key_in, value, Wq, bq, Wk, bk, Wv, bv):
    from concourse.bass_utils import run_bass_kernel_spmd

    nc = _get_nc()
    in_maps = build_in_maps({
        "query": query, "key_in": key_in, "value": value,
        "Wq": Wq, "bq": bq, "Wk": Wk, "bk": bk, "Wv": Wv, "bv": bv,
    })

    res = run_bass_kernel_spmd(nc, in_maps, core_ids=list(range(NCORES)))

    out = np.empty((B, S, D), np.float32)
    for c in range(NCORES):
        b, hg = divmod(c, 2)
        out[b, :, hg * CPC:(hg + 1) * CPC] = res.results[c]["out"].T
    return out


# revision 30
# speedup vs baseline: 2.0321x; 1.0634x over previous
"""nn_CrossAttention kernel v4 for 8x TRN2 NeuronCores.

Sharding: core c = (batch b = c//2, head-group hg = c%2 of 8 heads).

v4 design (v3 = 590us, v2 baseline = 746us):
 - v3 post-mortem: HAM oscillated 16x (41% of time at K=4/8). Root
   cause: at every phase boundary AV(c,0,h1) waited 3-5us for the norm
   chain (reciprocal 3.4us! -> DRAM bounce -> mul) to free an av PSUM
   bank -> MID re-throttle each phase. Also DVE ops pay a pipe-flush
   DRAIN, so 12 trick slots + norm saturated DVE.
 - Fixes:
   * Quick av->SBUF copies (ACT h0 / DVE h1, ~1us) free the av banks
     immediately at the boundary; norm runs off the copies.
   * reciprocal_approx_fast (~0.7us vs 3.4us, 51 ULP - plenty).
   * exp at 1024-wide over sc bank-PAIRS: [128,2,512] psum tiles, one
     exp op covers both heads of an i-slot. ACT 12 pairs + DVE 4 pairs
     (bit-trick exp, 25% of slots -> ~1e-2 total rel err).
   * Prologue: interleave kT t-passes with phase-0 QK blocks, qT t0
     right after wq; first QK at ~40us instead of 83us.
 - PSUM: vp 1 + sc 2x2 + av 3 = 8 banks.
"""

import json
import numpy as np

B, S, D, NH, HD = 4, 2048, 1024, 16, 64
CPC = 512          # cols per core = 8 heads * 64
NCORES = 8
NDT = D // 128     # 8 d-tiles
NP = CPC // 128    # 4 c-tiles (head pairs)
NSK = S // 128     # 16 sk-tiles
NJ = S // 512      # 4 sq chunks
SQC = 512          # sq chunk size
NPH = NP * NJ      # 16 phases

# bit-trick exp constants: bf16 bits = round(score * EXPA + EXPB)
# exp(s*0.125) = 2^(s*0.125*log2 e); bf16 bits = exp_field*128 + mantissa
EXPA = 0.125 * 1.4426950408889634 * 128.0   # 23.0831...
EXPB = 16256.0 - 7.5                        # 127*128 - sigma (sigma tuned)

# i-slot -> engine assignment: i % 4 == 2 -> DVE trick (4 of 16 pairs)
DVE_PAIRS = frozenset((2,))


# ---------------------------------------------------------------- drain fix
def _fix_module_json(bj: bytes) -> bytes:
    """This walrus build accepts at most ONE sync wait/update on CTRL-lowered
    instructions (Drain). Move extras onto EventSemaphore instructions."""
    d = json.loads(bj)
    counter = [0]

    def fix_block(b):
        out = []
        for inst in b.get("instructions", []):
            si = inst.get("sync_info") or {}
            ow = si.get("on_wait") or []
            ou = si.get("on_update") or []
            if (inst.get("opcode") not in
                    ("EventSemaphore", "Call", "RegisterMove",
                     "UnconditionalBranch", "ISA", "Drain") and len(ow) > 1):
                for w in ow[1:]:
                    counter[0] += 1
                    out.append({
                        "debug": inst.get("debug", 0),
                        "engine": inst["engine"],
                        "ins": [], "outs": [],
                        "name": f"synthmmw-{counter[0]}",
                        "opcode": "EventSemaphore",
                        "sync_info": {"on_update": [], "on_wait": [w]},
                    })
                inst["sync_info"] = {"on_update": ou, "on_wait": ow[:1]}
                out.append(inst)
                continue
            if inst.get("opcode") == "Drain" and (len(ow) > 1 or len(ou) > 1):
                for w in ow[1:]:
                    counter[0] += 1
                    out.append({
                        "debug": inst.get("debug", 0),
                        "engine": inst["engine"],
                        "ins": [], "outs": [],
                        "name": f"synthwait-{counter[0]}",
                        "opcode": "EventSemaphore",
                        "sync_info": {"on_update": [], "on_wait": [w]},
                    })
                inst["sync_info"] = {"on_update": ou[:1], "on_wait": ow[:1]}
                out.append(inst)
                for u in ou[1:]:
                    counter[0] += 1
                    out.append({
                        "debug": inst.get("debug", 0),
                        "engine": inst["engine"],
                        "ins": [], "outs": [],
                        "name": f"synthupd-{counter[0]}",
                        "opcode": "EventSemaphore",
                        "sync_info": {"on_update": [u], "on_wait": []},
                    })
            else:
                out.append(inst)
        b["instructions"] = out
        for sb in b.get("blocks", []):
            fix_block(sb)

    for fn in d.get("functions", []):
        for blk in fn.get("blocks", []):
            fix_block(blk)
    return json.dumps(d).encode()


def _install_drainfix():
    import concourse.bass as bass
    if getattr(bass.Bass, "_drainfix_installed", False):
        return
    orig = bass.Bass.to_json_bytes

    def patched(self):
        return _fix_module_json(orig(self))

    bass.Bass.to_json_bytes = patched
    bass.Bass._drainfix_installed = True


# ---------------------------------------------------------------- program
def _build_nc(reps=1):
    import concourse.bass as bass
    import concourse.mybir as mybir
    from concourse.tile import TileContext
    from contextlib import ExitStack

    f32 = mybir.dt.float32
    f32r = mybir.dt.float32r
    bf16 = mybir.dt.bfloat16
    i16 = mybir.dt.int16
    EXP = mybir.ActivationFunctionType.Exp
    MUL = mybir.AluOpType.mult
    ADD = mybir.AluOpType.add

    nc = bass.Bass("TRN2", num_devices=NCORES)

    xqT = nc.dram_tensor("xqT", [D, S], bf16, kind="ExternalInput")
    xkT = nc.dram_tensor("xkT", [D, S], bf16, kind="ExternalInput")
    xvT = nc.dram_tensor("xvT", [D, S], bf16, kind="ExternalInput")
    wq = nc.dram_tensor("wq", [D, CPC], bf16, kind="ExternalInput")
    wk = nc.dram_tensor("wk", [D, CPC], bf16, kind="ExternalInput")
    wv = nc.dram_tensor("wv", [D, CPC], bf16, kind="ExternalInput")
    bqd = nc.dram_tensor("bq", [CPC], f32, kind="ExternalInput")
    bkd = nc.dram_tensor("bk", [CPC], f32, kind="ExternalInput")
    bvd = nc.dram_tensor("bv", [CPC], f32, kind="ExternalInput")
    outd = nc.dram_tensor("out", [CPC, S], f32, kind="ExternalOutput")
    # scratch for the denominator-reciprocal broadcast bounce (stride-0
    # partition reads are only legal from DRAM)
    recscr = nc.dram_tensor("recscr", [2 * NPH, SQC], f32, kind="Internal")

    with ExitStack() as ctx:
        ctx.enter_context(nc.allow_low_precision(
            reason="qk in bf16 + bit-trick exp; matmul accumulates f32; "
                   "rel-err gate 2e-2"))
        tc = ctx.enter_context(TileContext(nc))
        sb = ctx.enter_context(tc.tile_pool(name="sb", bufs=1))
        ps = ctx.enter_context(tc.tile_pool(name="ps", bufs=1, space="PSUM"))

        # ---- resident weights / constants (wk + xk first: critical) ----
        wk_sb = sb.tile([128, NDT, CPC], bf16, name="wk_sb")
        for dd in range(NDT):
            nc.sync.dma_start(out=wk_sb[:, dd, :],
                              in_=wk[dd * 128:(dd + 1) * 128, :])
        bk_sb = sb.tile([128, NP], f32, name="bk_sb")
        nc.sync.dma_start(out=bk_sb, in_=bkd.rearrange("(p c) -> c p", p=NP))
        bq_sb = sb.tile([128, NP], f32, name="bq_sb")
        nc.sync.dma_start(out=bq_sb, in_=bqd.rearrange("(p c) -> c p", p=NP))

        # persistent activation tiles
        qT = [sb.tile([128, S], bf16, name=f"qT{p}") for p in range(NP)]
        kT = [sb.tile([128, S], bf16, name=f"kT{p}") for p in range(NP)]
        v_aug = sb.tile([128, NSK, 8, 65], bf16, name="v_aug")

        # ---- helpers ----
        def load_x2(x_dram, u):
            """8 [128, 1024] bf16 tiles covering t-chunks 2u, 2u+1 (2KB
            per-partition DMA lines)."""
            xts = []
            for dd in range(NDT):
                x_t = sb.tile([128, 2 * SQC], bf16, tag="xs", bufs=12,
                              name=f"x_{u}_{dd}")
                nc.sync.dma_start(
                    out=x_t,
                    in_=x_dram[dd * 128:(dd + 1) * 128,
                               u * 2 * SQC:(u + 1) * 2 * SQC])
                xts.append(x_t)
            return xts

        def load_x1(x_dram, t):
            """8 single-t [128, 512] bf16 tiles (1KB lines; used for the qT
            passes so the critical path to the first QK is shorter)."""
            xts = []
            for dd in range(NDT):
                x_t = sb.tile([128, SQC], bf16, tag="xs1", bufs=12,
                              name=f"x1_{t}_{dd}")
                nc.sync.dma_start(
                    out=x_t,
                    in_=x_dram[dd * 128:(dd + 1) * 128,
                               t * SQC:(t + 1) * SQC])
                xts.append(x_t)
            return xts

        def emit_proj(t, plist, w_sb, bias_sb, dst, xts, single=False):
            """dst[p][:, t*512:(t+1)*512] = (x @ W + b).T chunk (bf16).
            xts are the [128, 1024] pair tiles for u = t//2 (or single-t
            [128, 512] tiles if single)."""
            xo = 0 if single else (t % 2) * SQC
            for p in plist:
                pr = ps.tile([128, SQC], f32, tag="vp", bufs=2,
                             name=f"prj_{t}_{p}")
                for dd in range(NDT):
                    nc.tensor.matmul(
                        pr[:, :],
                        w_sb[:, dd, p * 128:(p + 1) * 128],
                        xts[dd][:, xo:xo + SQC],
                        start=(dd == 0), stop=(dd == NDT - 1))
                nc.vector.tensor_scalar_add(
                    dst[p][:, t * SQC:(t + 1) * SQC], pr[:, :],
                    bias_sb[:, p:p + 1])

        xv_group = {}  # g -> list of 8 [128, 512] bf16 tiles (tt 4g..4g+3)

        def load_xv(g):
            xvt = []
            for dd in range(NDT):
                xv_t = sb.tile([128, 4 * 128], bf16, tag="xv", bufs=12,
                               name=f"xv_{g}_{dd}")
                nc.sync.dma_start(
                    out=xv_t,
                    in_=xvT[dd * 128:(dd + 1) * 128,
                            g * 512:(g + 1) * 512])
                xvt.append(xv_t)
            xv_group[g] = xvt

        def emit_proj_v(tt):
            """v_aug[:, tt, h, 0:64] = (xv @ Wv + bv) rows tt*128.., bf16."""
            xvt = xv_group[tt // 4]
            co = (tt % 4) * 128
            pv = ps.tile([128, CPC], f32, tag="vp", bufs=2, name=f"pv_{tt}")
            for dd in range(NDT):
                nc.tensor.matmul(
                    pv[:, :], xvt[dd][:, co:co + 128], wv_sb[:, dd, :],
                    start=(dd == 0), stop=(dd == NDT - 1))
            nc.vector.tensor_add(
                v_aug[:, tt, :, 0:64],
                pv.rearrange("c (h d) -> c h d", h=8),
                bv_bc.rearrange("c (h d) -> c h d", h=8))
            if tt % 4 == 3:
                xv_group.pop(tt // 4)

        alpha = {}     # (c, i) -> [128, 2, 512] bf16 pair tile
        av_tiles = {}  # c -> {h: psum tile [65, 512]}

        def emit_qk_i(c, i):
            p, j = divmod(c, NJ)
            scp = ps.tile([128, 2, SQC], f32, tag="sc", bufs=2,
                          name=f"sc_{c}_{i}")
            for h in range(2):
                nc.tensor.matmul(
                    scp[:, h, :],
                    kT[p][h * 64:(h + 1) * 64, i * 128:(i + 1) * 128],
                    qT[p][h * 64:(h + 1) * 64, j * SQC:(j + 1) * SQC],
                    start=True, stop=True)
            a_p = sb.tile([128, 2, SQC], bf16, tag="alpha", bufs=18,
                          name=f"al_{c}_{i}")
            if i % 4 in DVE_PAIRS:
                nc.vector.tensor_scalar(
                    a_p.bitcast(i16), scp[:, :, :], EXPA, EXPB, MUL, ADD)
            else:
                nc.scalar.activation(a_p[:, :, :], scp[:, :, :], EXP,
                                     scale=0.125)
            alpha[(c, i)] = a_p

        def emit_av_i(c, i):
            p, j = divmod(c, NJ)
            if i == 0:
                av_tiles[c] = {
                    h: ps.tile([65, SQC], f32, tag="av", bufs=2,
                               name=f"av_{c}_{h}")
                    for h in range(2)}
            a_p = alpha[(c, i)]
            for h in range(2):
                nc.tensor.matmul(
                    av_tiles[c][h][:, :],
                    v_aug[:, i, 2 * p + h, :],
                    a_p[:, h, :],
                    start=(i == 0), stop=(i == NSK - 1))
            alpha.pop((c, i))

        norm_state = {}  # c -> (avs, recB)

        def emit_norm_head(c):
            """Free the av PSUM banks FAST: both copies on DVE (whose last
            trick pair is slot 14, so it's free at the boundary). Then the
            denominator bounce: raw row -> DRAM -> [64,8] spread so the
            6-cycles/elem DVE reciprocal costs 8 elems/lane -> DRAM ->
            [64,512] broadcast."""
            avt = av_tiles.pop(c)
            avs, recB = {}, {}
            for h in range(2):
                avs[h] = sb.tile([65, SQC], f32, tag="avs", bufs=4,
                                 name=f"avs_{c}_{h}")
                nc.vector.tensor_copy(avs[h][:, :], avt[h][:, :])
            for h in range(2):
                slot = 2 * c + h
                nc.sync.dma_start(out=recscr[slot:slot + 1, :],
                                  in_=avs[h][64:65, :])
                _r = recscr[slot:slot + 1, :]
                den8 = sb.tile([64, 8], f32, tag="den8", bufs=4,
                               name=f"den8_{c}_{h}")
                nc.sync.dma_start(
                    out=den8,
                    in_=bass.AP(tensor=_r.tensor, offset=_r.offset,
                                ap=[[8, 64], [1, 8]]))
                rec8 = sb.tile([64, 8], f32, tag="rec8", bufs=4,
                               name=f"rec8_{c}_{h}")
                nc.vector.reciprocal(rec8[:, :], den8[:, :])
                nc.sync.dma_start(
                    out=bass.AP(tensor=_r.tensor, offset=_r.offset,
                                ap=[[8, 64], [1, 8]]),
                    in_=rec8[:, :])
                recB[h] = sb.tile([64, SQC], f32, tag="recB", bufs=4,
                                  name=f"recB_{c}_{h}")
                nc.sync.dma_start(
                    out=recB[h],
                    in_=bass.AP(tensor=_r.tensor, offset=_r.offset,
                                ap=[[0, 64]] + list(_r.ap)[1:]))
            norm_state[c] = (avs, recB)

        def emit_norm_tail(c):
            """Normalize-multiply on the idle Pool engine (all-SBUF), DMA
            out. Emitted mid-next-phase so the recB bounce has landed."""
            p, j = divmod(c, NJ)
            avs, recB = norm_state.pop(c)
            for h in range(2):
                r0 = (2 * p + h) * 64
                cx = sb.tile([64, SQC], f32, tag="cx", bufs=4,
                             name=f"cx_{c}_{h}")
                nc.gpsimd.tensor_mul(cx[:, :], avs[h][0:64, :], recB[h][:, :])
                nc.sync.dma_start(
                    out=outd[r0:r0 + 64, j * SQC:(j + 1) * SQC],
                    in_=cx[:, :])

        # late-loaded residents (emitted after critical-path DMAs above,
        # but data only needed from mid-prologue onwards)
        wq_sb = sb.tile([128, NDT, CPC], bf16, name="wq_sb")
        wv_sb = sb.tile([128, NDT, CPC], bf16, name="wv_sb")
        bv_bc = sb.tile([128, CPC], f32, name="bv_bc")

        def _emit_wq():
            for dd in range(NDT):
                nc.sync.dma_start(
                    out=wq_sb[:, dd, :],
                    in_=wq[dd * 128:(dd + 1) * 128, :])

        def _emit_wv():
            for dd in range(NDT):
                nc.sync.dma_start(
                    out=wv_sb[:, dd, :],
                    in_=wv[dd * 128:(dd + 1) * 128, :])
            _bva = bvd[:]
            nc.sync.dma_start(
                out=bv_bc,
                in_=bass.AP(tensor=_bva.tensor, offset=_bva.offset,
                            ap=[[0, 128]] + list(_bva.ap)))
            nc.gpsimd.memset(v_aug[:, :, :, 64:65], 1.0)

        # ---- emission schedule ----
        def _emit_all():
            # prologue (x/W in bf16, [128,1024] x pair-tiles): kT t0/t1 ->
            # qT t0/t1 -> phase-0 QK 0..7 -> kT t2/t3 -> QK 8..15 ->
            # qT t2/t3 -> wv. wq DMA right after xk u0 so the first qT
            # chain isn't blocked.
            xk0 = load_x2(xkT, 0)
            _emit_wq()
            emit_proj(0, [0, 1, 2, 3], wk_sb, bk_sb, kT, xk0)
            emit_proj(1, [0, 1, 2, 3], wk_sb, bk_sb, kT, xk0)
            xq0 = load_x1(xqT, 0)
            emit_proj(0, [0, 1, 2, 3], wq_sb, bq_sb, qT, xq0, single=True)
            for i in range(4):
                emit_qk_i(0, i)
            xk1 = load_x2(xkT, 1)
            emit_proj(2, [0, 1, 2, 3], wk_sb, bk_sb, kT, xk1)
            for i in range(4, 8):
                emit_qk_i(0, i)
            emit_proj(3, [0, 1, 2, 3], wk_sb, bk_sb, kT, xk1)
            for i in range(8, 12):
                emit_qk_i(0, i)
            xq1 = load_x1(xqT, 1)
            emit_proj(1, [0, 1, 2, 3], wq_sb, bq_sb, qT, xq1, single=True)
            for i in range(12, 16):
                emit_qk_i(0, i)
            xq2 = load_x1(xqT, 2)
            emit_proj(2, [0, 1, 2, 3], wq_sb, bq_sb, qT, xq2, single=True)
            xq3 = load_x1(xqT, 3)
            emit_proj(3, [0, 1, 2, 3], wq_sb, bq_sb, qT, xq3, single=True)
            _emit_wv()

            # phase 1: v-projection (xv DMA paced) interleaved with QK(1)
            # and AV(0) (AV(0,i) needs v_aug[:, i] just computed).
            for i in range(NSK):
                if i == 0:
                    load_xv(0)
                if i % 4 == 1 and i // 4 + 1 < 4:
                    load_xv(i // 4 + 1)
                emit_proj_v(i)
                emit_qk_i(1, i)
                emit_av_i(0, i)
            emit_norm_head(0)

            # phases 2..15: pure QK/AV; norm tail of phase c-2 mid-phase
            # (after its recB bounce has landed), norm head of c-1 at end.
            for c in range(2, NPH):
                for i in range(NSK):
                    emit_qk_i(c, i)
                    emit_av_i(c - 1, i)
                    if i == 7 and c >= 2:
                        emit_norm_tail(c - 2)
                emit_norm_head(c - 1)

            # epilogue
            for i in range(NSK):
                emit_av_i(NPH - 1, i)
                if i == 7:
                    emit_norm_tail(NPH - 2)
            emit_norm_head(NPH - 1)
            emit_norm_tail(NPH - 1)

        for _rep in range(reps):
            _emit_all()

    return nc


_NC_BY_REPS = {}


def _get_nc(reps=1):
    if reps not in _NC_BY_REPS:
        _install_drainfix()
        _NC_BY_REPS[reps] = _build_nc(reps)
    return _NC_BY_REPS[reps]


# ---------------------------------------------------------------- entry
def build_in_maps(inputs):
    import ml_dtypes

    bf16 = ml_dtypes.bfloat16
    query = np.asarray(inputs["query"], np.float32)
    key_in = np.asarray(inputs["key_in"], np.float32)
    value = np.asarray(inputs["value"], np.float32)
    Wq = np.asarray(inputs["Wq"], np.float32)
    Wk = np.asarray(inputs["Wk"], np.float32)
    Wv = np.asarray(inputs["Wv"], np.float32)
    bq = np.asarray(inputs["bq"], np.float32)
    bk = np.asarray(inputs["bk"], np.float32)
    bv = np.asarray(inputs["bv"], np.float32)

    in_maps = []
    for c in range(NCORES):
        b, hg = divmod(c, 2)
        cols = slice(hg * CPC, (hg + 1) * CPC)
        in_maps.append({
            "xqT": np.ascontiguousarray(query[b].T.astype(bf16)),
            "xkT": np.ascontiguousarray(key_in[b].T.astype(bf16)),
            "xvT": np.ascontiguousarray(value[b].T.astype(bf16)),
            "wq": np.ascontiguousarray(Wq[:, cols].astype(bf16)),
            "wk": np.ascontiguousarray(Wk[:, cols].astype(bf16)),
            "wv": np.ascontiguousarray(Wv[:, cols].astype(bf16)),
            "bq": np.ascontiguousarray(bq[cols]),
            "bk": np.ascontiguousarray(bk[cols]),
            "bv": np.ascontiguousarray(bv[cols]),
        })
    return in_maps


def kernel(query, key_in, value, Wq, bq, Wk, bk, Wv, bv):
    from concourse.bass_utils import run_bass_kernel_spmd

    nc = _get_nc()
    in_maps = build_in_maps({
        "query": query, "key_in": key_in, "value": value,
        "Wq": Wq, "bq": bq, "Wk": Wk, "bk": bk, "Wv": Wv, "bv": bv,
    })

    res = run_bass_kernel_spmd(nc, in_maps, core_ids=list(range(NCORES)))

    out = np.empty((B, S, D), np.float32)
    for c in range(NCORES):
        b, hg = divmod(c, 2)
        out[b, :, hg * CPC:(hg + 1) * CPC] = res.results[c]["out"].T
    return out


# revision 38
# speedup vs baseline: 2.0562x; 1.0118x over previous
"""nn_CrossAttention kernel v4 for 8x TRN2 NeuronCores.

Sharding: core c = (batch b = c//2, head-group hg = c%2 of 8 heads).

v4 design (v3 = 590us, v2 baseline = 746us):
 - v3 post-mortem: HAM oscillated 16x (41% of time at K=4/8). Root
   cause: at every phase boundary AV(c,0,h1) waited 3-5us for the norm
   chain (reciprocal 3.4us! -> DRAM bounce -> mul) to free an av PSUM
   bank -> MID re-throttle each phase. Also DVE ops pay a pipe-flush
   DRAIN, so 12 trick slots + norm saturated DVE.
 - Fixes:
   * Quick av->SBUF copies (ACT h0 / DVE h1, ~1us) free the av banks
     immediately at the boundary; norm runs off the copies.
   * reciprocal_approx_fast (~0.7us vs 3.4us, 51 ULP - plenty).
   * exp at 1024-wide over sc bank-PAIRS: [128,2,512] psum tiles, one
     exp op covers both heads of an i-slot. ACT 12 pairs + DVE 4 pairs
     (bit-trick exp, 25% of slots -> ~1e-2 total rel err).
   * Prologue: interleave kT t-passes with phase-0 QK blocks, qT t0
     right after wq; first QK at ~40us instead of 83us.
 - PSUM: vp 1 + sc 2x2 + av 3 = 8 banks.
"""

import json
import numpy as np

B, S, D, NH, HD = 4, 2048, 1024, 16, 64
CPC = 512          # cols per core = 8 heads * 64
NCORES = 8
NDT = D // 128     # 8 d-tiles
NP = CPC // 128    # 4 c-tiles (head pairs)
NSK = S // 128     # 16 sk-tiles
NJ = S // 512      # 4 sq chunks
SQC = 512          # sq chunk size
NPH = NP * NJ      # 16 phases

# bit-trick exp constants: bf16 bits = round(score * EXPA + EXPB)
# exp(s*0.125) = 2^(s*0.125*log2 e); bf16 bits = exp_field*128 + mantissa
EXPA = 0.125 * 1.4426950408889634 * 128.0   # 23.0831...
EXPB = 16256.0 - 7.5                        # 127*128 - sigma (sigma tuned)

# i-slot -> engine assignment: DVE trick pairs (5 of 16 per phase)
DVE_PAIRS_I = frozenset((2, 5, 8, 11, 14))


# ---------------------------------------------------------------- drain fix
def _fix_module_json(bj: bytes) -> bytes:
    """This walrus build accepts at most ONE sync wait/update on CTRL-lowered
    instructions (Drain). Move extras onto EventSemaphore instructions."""
    d = json.loads(bj)
    counter = [0]

    def fix_block(b):
        out = []
        for inst in b.get("instructions", []):
            si = inst.get("sync_info") or {}
            ow = si.get("on_wait") or []
            ou = si.get("on_update") or []
            if (inst.get("opcode") not in
                    ("EventSemaphore", "Call", "RegisterMove",
                     "UnconditionalBranch", "ISA", "Drain") and len(ow) > 1):
                for w in ow[1:]:
                    counter[0] += 1
                    out.append({
                        "debug": inst.get("debug", 0),
                        "engine": inst["engine"],
                        "ins": [], "outs": [],
                        "name": f"synthmmw-{counter[0]}",
                        "opcode": "EventSemaphore",
                        "sync_info": {"on_update": [], "on_wait": [w]},
                    })
                inst["sync_info"] = {"on_update": ou, "on_wait": ow[:1]}
                out.append(inst)
                continue
            if inst.get("opcode") == "Drain" and (len(ow) > 1 or len(ou) > 1):
                for w in ow[1:]:
                    counter[0] += 1
                    out.append({
                        "debug": inst.get("debug", 0),
                        "engine": inst["engine"],
                        "ins": [], "outs": [],
                        "name": f"synthwait-{counter[0]}",
                        "opcode": "EventSemaphore",
                        "sync_info": {"on_update": [], "on_wait": [w]},
                    })
                inst["sync_info"] = {"on_update": ou[:1], "on_wait": ow[:1]}
                out.append(inst)
                for u in ou[1:]:
                    counter[0] += 1
                    out.append({
                        "debug": inst.get("debug", 0),
                        "engine": inst["engine"],
                        "ins": [], "outs": [],
                        "name": f"synthupd-{counter[0]}",
                        "opcode": "EventSemaphore",
                        "sync_info": {"on_update": [u], "on_wait": []},
                    })
            else:
                out.append(inst)
        b["instructions"] = out
        for sb in b.get("blocks", []):
            fix_block(sb)

    for fn in d.get("functions", []):
        for blk in fn.get("blocks", []):
            fix_block(blk)
    return json.dumps(d).encode()


def _install_drainfix():
    import concourse.bass as bass
    if getattr(bass.Bass, "_drainfix_installed", False):
        return
    orig = bass.Bass.to_json_bytes

    def patched(self):
        return _fix_module_json(orig(self))

    bass.Bass.to_json_bytes = patched
    bass.Bass._drainfix_installed = True


# ---------------------------------------------------------------- program
def _build_nc(reps=1):
    import concourse.bass as bass
    import concourse.mybir as mybir
    from concourse.tile import TileContext
    from contextlib import ExitStack

    f32 = mybir.dt.float32
    f32r = mybir.dt.float32r
    bf16 = mybir.dt.bfloat16
    i16 = mybir.dt.int16
    EXP = mybir.ActivationFunctionType.Exp
    MUL = mybir.AluOpType.mult
    ADD = mybir.AluOpType.add

    nc = bass.Bass("TRN2", num_devices=NCORES)

    xqT = nc.dram_tensor("xqT", [D, S], bf16, kind="ExternalInput")
    xkT = nc.dram_tensor("xkT", [D, S], bf16, kind="ExternalInput")
    xvT = nc.dram_tensor("xvT", [D, S], bf16, kind="ExternalInput")
    wq = nc.dram_tensor("wq", [D, CPC], bf16, kind="ExternalInput")
    wk = nc.dram_tensor("wk", [D, CPC], bf16, kind="ExternalInput")
    wv = nc.dram_tensor("wv", [D, CPC], bf16, kind="ExternalInput")
    bqd = nc.dram_tensor("bq", [CPC], f32, kind="ExternalInput")
    bkd = nc.dram_tensor("bk", [CPC], f32, kind="ExternalInput")
    bvd = nc.dram_tensor("bv", [CPC], f32, kind="ExternalInput")
    outd = nc.dram_tensor("out", [CPC, S], f32, kind="ExternalOutput")
    # scratch for the denominator-reciprocal broadcast bounce (stride-0
    # partition reads are only legal from DRAM)
    recscr = nc.dram_tensor("recscr", [2 * NPH, SQC], f32, kind="Internal")

    with ExitStack() as ctx:
        ctx.enter_context(nc.allow_low_precision(
            reason="qk in bf16 + bit-trick exp; matmul accumulates f32; "
                   "rel-err gate 2e-2"))
        tc = ctx.enter_context(TileContext(nc))
        sb = ctx.enter_context(tc.tile_pool(name="sb", bufs=1))
        ps = ctx.enter_context(tc.tile_pool(name="ps", bufs=1, space="PSUM"))

        # ---- resident weights / constants (wk + xk first: critical) ----
        wk_sb = sb.tile([128, NDT, CPC], bf16, name="wk_sb")
        for dd in range(NDT):
            nc.sync.dma_start(out=wk_sb[:, dd, :],
                              in_=wk[dd * 128:(dd + 1) * 128, :])
        bk_sb = sb.tile([128, NP], f32, name="bk_sb")
        nc.sync.dma_start(out=bk_sb, in_=bkd.rearrange("(p c) -> c p", p=NP))
        bq_sb = sb.tile([128, NP], f32, name="bq_sb")
        nc.sync.dma_start(out=bq_sb, in_=bqd.rearrange("(p c) -> c p", p=NP))

        # persistent activation tiles
        qT = [sb.tile([128, S], bf16, name=f"qT{p}") for p in range(NP)]
        kT = [sb.tile([128, S], bf16, name=f"kT{p}") for p in range(NP)]
        v_aug = sb.tile([128, NSK, 8, 65], bf16, name="v_aug")

        # ---- helpers ----
        def load_x2(x_dram, u):
            """8 [128, 1024] bf16 tiles covering t-chunks 2u, 2u+1 (2KB
            per-partition DMA lines)."""
            xts = []
            for dd in range(NDT):
                x_t = sb.tile([128, 2 * SQC], bf16, tag="xs", bufs=10,
                              name=f"x_{u}_{dd}")
                nc.sync.dma_start(
                    out=x_t,
                    in_=x_dram[dd * 128:(dd + 1) * 128,
                               u * 2 * SQC:(u + 1) * 2 * SQC])
                xts.append(x_t)
            return xts

        def load_x1(x_dram, t):
            """8 single-t [128, 512] bf16 tiles (1KB lines; used for the qT
            passes so the critical path to the first QK is shorter)."""
            xts = []
            for dd in range(NDT):
                x_t = sb.tile([128, SQC], bf16, tag="xs1", bufs=16,
                              name=f"x1_{t}_{dd}")
                nc.sync.dma_start(
                    out=x_t,
                    in_=x_dram[dd * 128:(dd + 1) * 128,
                               t * SQC:(t + 1) * SQC])
                xts.append(x_t)
            return xts

        def emit_proj(t, plist, w_sb, bias_sb, dst, xts, single=False):
            """dst[p][:, t*512:(t+1)*512] = (x @ W + b).T chunk (bf16).
            xts are the [128, 1024] pair tiles for u = t//2 (or single-t
            [128, 512] tiles if single)."""
            xo = 0 if single else (t % 2) * SQC
            for p in plist:
                pr = ps.tile([128, SQC], f32, tag="vp", bufs=2,
                             name=f"prj_{t}_{p}")
                for dd in range(NDT):
                    nc.tensor.matmul(
                        pr[:, :],
                        w_sb[:, dd, p * 128:(p + 1) * 128],
                        xts[dd][:, xo:xo + SQC],
                        start=(dd == 0), stop=(dd == NDT - 1))
                nc.vector.tensor_scalar_add(
                    dst[p][:, t * SQC:(t + 1) * SQC], pr[:, :],
                    bias_sb[:, p:p + 1])

        xv_group = {}  # g -> list of 8 [128, 512] bf16 tiles (tt 4g..4g+3)

        def load_xv(g):
            xvt = []
            for dd in range(NDT):
                xv_t = sb.tile([128, 4 * 128], bf16, tag="xv", bufs=12,
                               name=f"xv_{g}_{dd}")
                nc.sync.dma_start(
                    out=xv_t,
                    in_=xvT[dd * 128:(dd + 1) * 128,
                            g * 512:(g + 1) * 512])
                xvt.append(xv_t)
            xv_group[g] = xvt

        def emit_proj_v(tt):
            """v_aug[:, tt, h, 0:64] = (xv @ Wv + bv) rows tt*128.., bf16."""
            xvt = xv_group[tt // 4]
            co = (tt % 4) * 128
            pv = ps.tile([128, CPC], f32, tag="vp", bufs=2, name=f"pv_{tt}")
            for dd in range(NDT):
                nc.tensor.matmul(
                    pv[:, :], xvt[dd][:, co:co + 128], wv_sb[:, dd, :],
                    start=(dd == 0), stop=(dd == NDT - 1))
            nc.vector.tensor_add(
                v_aug[:, tt, :, 0:64],
                pv.rearrange("c (h d) -> c h d", h=8),
                bv_bc.rearrange("c (h d) -> c h d", h=8))
            if tt % 4 == 3:
                xv_group.pop(tt // 4)

        alpha = {}     # (c, i) -> [128, 2, 512] bf16 pair tile
        av_tiles = {}  # c -> {h: psum tile [65, 512]}

        def emit_qk_i(c, i):
            p, j = divmod(c, NJ)
            scp = ps.tile([128, 2, SQC], f32, tag="sc", bufs=2,
                          name=f"sc_{c}_{i}")
            for h in range(2):
                nc.tensor.matmul(
                    scp[:, h, :],
                    kT[p][h * 64:(h + 1) * 64, i * 128:(i + 1) * 128],
                    qT[p][h * 64:(h + 1) * 64, j * SQC:(j + 1) * SQC],
                    start=True, stop=True)
            a_p = sb.tile([128, 2, SQC], bf16, tag="alpha", bufs=18,
                          name=f"al_{c}_{i}")
            if i in DVE_PAIRS_I:
                nc.vector.tensor_scalar(
                    a_p.bitcast(i16), scp[:, :, :], EXPA, EXPB, MUL, ADD)
            else:
                nc.scalar.activation(a_p[:, :, :], scp[:, :, :], EXP,
                                     scale=0.125)
            alpha[(c, i)] = a_p

        def emit_av_i(c, i):
            p, j = divmod(c, NJ)
            if i == 0:
                av_tiles[c] = {
                    h: ps.tile([65, SQC], f32, tag="av", bufs=2,
                               name=f"av_{c}_{h}")
                    for h in range(2)}
            a_p = alpha[(c, i)]
            for h in range(2):
                nc.tensor.matmul(
                    av_tiles[c][h][:, :],
                    v_aug[:, i, 2 * p + h, :],
                    a_p[:, h, :],
                    start=(i == 0), stop=(i == NSK - 1))
            alpha.pop((c, i))

        norm_state = {}  # c -> (avs, recB)

        def emit_norm_head(c):
            """Free the av PSUM banks FAST: both copies on DVE (whose last
            trick pair is slot 14, so it's free at the boundary). Then the
            denominator bounce: raw row -> DRAM -> [64,8] spread so the
            6-cycles/elem DVE reciprocal costs 8 elems/lane -> DRAM ->
            [64,512] broadcast."""
            avt = av_tiles.pop(c)
            avs, recB = {}, {}
            for h in range(2):
                avs[h] = sb.tile([65, SQC], f32, tag="avs", bufs=4,
                                 name=f"avs_{c}_{h}")
                nc.vector.tensor_copy(avs[h][:, :], avt[h][:, :])
            for h in range(2):
                slot = 2 * c + h
                nc.sync.dma_start(out=recscr[slot:slot + 1, :],
                                  in_=avs[h][64:65, :])
                _r = recscr[slot:slot + 1, :]
                den8 = sb.tile([64, 8], f32, tag="den8", bufs=4,
                               name=f"den8_{c}_{h}")
                nc.sync.dma_start(
                    out=den8,
                    in_=bass.AP(tensor=_r.tensor, offset=_r.offset,
                                ap=[[8, 64], [1, 8]]))
                rec8 = sb.tile([64, 8], f32, tag="rec8", bufs=4,
                               name=f"rec8_{c}_{h}")
                nc.vector.reciprocal(rec8[:, :], den8[:, :])
                nc.sync.dma_start(
                    out=bass.AP(tensor=_r.tensor, offset=_r.offset,
                                ap=[[8, 64], [1, 8]]),
                    in_=rec8[:, :])
                recB[h] = sb.tile([64, SQC], f32, tag="recB", bufs=4,
                                  name=f"recB_{c}_{h}")
                nc.sync.dma_start(
                    out=recB[h],
                    in_=bass.AP(tensor=_r.tensor, offset=_r.offset,
                                ap=[[0, 64]] + list(_r.ap)[1:]))
            norm_state[c] = (avs, recB)

        def _emit_norm_last(c, h):
            """Latency-optimized norm for the final chunk: direct DVE
            reciprocal of the row (no [64,8] spread), ONE DRAM bounce for
            the broadcast, DVE multiply."""
            p, j = divmod(c, NJ)
            avt = av_tiles[c]
            r0 = (2 * p + h) * 64
            slot = 2 * c + h
            avs = sb.tile([65, SQC], f32, tag="avs", bufs=4,
                          name=f"avsL_{h}")
            nc.vector.tensor_copy(avs[:, :], avt[h][:, :])
            rec = sb.tile([1, SQC], f32, tag="recL", bufs=2,
                          name=f"recL_{h}")
            nc.vector.reciprocal(rec[:, :], avs[64:65, :])
            nc.sync.dma_start(out=recscr[slot:slot + 1, :], in_=rec[:, :])
            _r = recscr[slot:slot + 1, :]
            recB = sb.tile([64, SQC], f32, tag="recB", bufs=4,
                           name=f"recBL_{h}")
            nc.sync.dma_start(
                out=recB,
                in_=bass.AP(tensor=_r.tensor, offset=_r.offset,
                            ap=[[0, 64]] + list(_r.ap)[1:]))
            cx = sb.tile([64, SQC], f32, tag="cx", bufs=4,
                         name=f"cxL_{h}")
            nc.vector.tensor_mul(cx[:, :], avs[0:64, :], recB[:, :])
            nc.sync.dma_start(
                out=outd[r0:r0 + 64, j * SQC:(j + 1) * SQC],
                in_=cx[:, :])

        def emit_norm_tail(c):
            """Normalize-multiply on the idle Pool engine (all-SBUF), DMA
            out. Emitted mid-next-phase so the recB bounce has landed."""
            p, j = divmod(c, NJ)
            avs, recB = norm_state.pop(c)
            for h in range(2):
                r0 = (2 * p + h) * 64
                cx = sb.tile([64, SQC], f32, tag="cx", bufs=4,
                             name=f"cx_{c}_{h}")
                nc.gpsimd.tensor_mul(cx[:, :], avs[h][0:64, :], recB[h][:, :])
                nc.sync.dma_start(
                    out=outd[r0:r0 + 64, j * SQC:(j + 1) * SQC],
                    in_=cx[:, :])

        # late-loaded residents (emitted after critical-path DMAs above,
        # but data only needed from mid-prologue onwards)
        wq_sb = sb.tile([128, NDT, CPC], bf16, name="wq_sb")
        wv_sb = sb.tile([128, NDT, CPC], bf16, name="wv_sb")
        bv_bc = sb.tile([128, CPC], f32, name="bv_bc")

        def _emit_wq():
            for dd in range(NDT):
                nc.sync.dma_start(
                    out=wq_sb[:, dd, :],
                    in_=wq[dd * 128:(dd + 1) * 128, :])

        def _emit_wv():
            for dd in range(NDT):
                nc.sync.dma_start(
                    out=wv_sb[:, dd, :],
                    in_=wv[dd * 128:(dd + 1) * 128, :])
            _bva = bvd[:]
            nc.sync.dma_start(
                out=bv_bc,
                in_=bass.AP(tensor=_bva.tensor, offset=_bva.offset,
                            ap=[[0, 128]] + list(_bva.ap)))
            nc.gpsimd.memset(v_aug[:, :, :, 64:65], 1.0)

        # ---- emission schedule ----
        def _emit_all():
            # prologue (x/W in bf16, [128,1024] x pair-tiles): kT t0/t1 ->
            # qT t0/t1 -> phase-0 QK 0..7 -> kT t2/t3 -> QK 8..15 ->
            # qT t2/t3 -> wv. wq DMA right after xk u0 so the first qT
            # chain isn't blocked.
            xkt0 = load_x1(xkT, 0)
            _emit_wq()
            emit_proj(0, [0, 1, 2, 3], wk_sb, bk_sb, kT, xkt0, single=True)
            xq0 = load_x1(xqT, 0)
            emit_proj(0, [0, 1, 2, 3], wq_sb, bq_sb, qT, xq0, single=True)
            for i in range(4):
                emit_qk_i(0, i)
            xkt1 = load_x1(xkT, 1)
            emit_proj(1, [0, 1, 2, 3], wk_sb, bk_sb, kT, xkt1, single=True)
            xk1 = load_x2(xkT, 1)
            emit_proj(2, [0, 1, 2, 3], wk_sb, bk_sb, kT, xk1)
            for i in range(4, 8):
                emit_qk_i(0, i)
            emit_proj(3, [0, 1, 2, 3], wk_sb, bk_sb, kT, xk1)
            for i in range(8, 12):
                emit_qk_i(0, i)
            xq1 = load_x1(xqT, 1)
            emit_proj(1, [0, 1, 2, 3], wq_sb, bq_sb, qT, xq1, single=True)
            for i in range(12, 16):
                emit_qk_i(0, i)
            xq2 = load_x1(xqT, 2)
            emit_proj(2, [0, 1, 2, 3], wq_sb, bq_sb, qT, xq2, single=True)
            xq3 = load_x1(xqT, 3)
            emit_proj(3, [0, 1, 2, 3], wq_sb, bq_sb, qT, xq3, single=True)
            _emit_wv()

            # phase 1: v-projection (xv DMA paced) interleaved with QK(1)
            # and AV(0) (AV(0,i) needs v_aug[:, i] just computed).
            for i in range(NSK):
                if i == 0:
                    load_xv(0)
                if i % 4 == 1 and i // 4 + 1 < 4:
                    load_xv(i // 4 + 1)
                emit_proj_v(i)
                emit_qk_i(1, i)
                emit_av_i(0, i)
            emit_norm_head(0)

            # phases 2..15: pure QK/AV; norm tail of phase c-2 mid-phase
            # (after its recB bounce has landed), norm head of c-1 at end.
            for c in range(2, NPH):
                for i in range(NSK):
                    emit_qk_i(c, i)
                    emit_av_i(c - 1, i)
                    if i == 7 and c >= 2:
                        emit_norm_tail(c - 2)
                emit_norm_head(c - 1)

            # epilogue: run h0's AV chain first so norm(15) h0 (copy +
            # direct reciprocal + single bounce + DVE mul) overlaps h1's
            # chain; latency-optimized norm path for the final chunk.
            c = NPH - 1
            p, j = divmod(c, NJ)
            av_tiles[c] = {
                h: ps.tile([65, SQC], f32, tag="av", bufs=2,
                           name=f"av_{c}_{h}")
                for h in range(2)}
            for h in range(2):
                for i in range(NSK):
                    nc.tensor.matmul(
                        av_tiles[c][h][:, :],
                        v_aug[:, i, 2 * p + h, :],
                        alpha[(c, i)][:, h, :],
                        start=(i == 0), stop=(i == NSK - 1))
                if h == 0:
                    emit_norm_tail(NPH - 2)
                    _emit_norm_last(c, 0)
            for i in range(NSK):
                alpha.pop((c, i))
            _emit_norm_last(c, 1)

        for _rep in range(reps):
            _emit_all()

    return nc


_NC_BY_REPS = {}


def _get_nc(reps=1):
    if reps not in _NC_BY_REPS:
        _install_drainfix()
        _NC_BY_REPS[reps] = _build_nc(reps)
    return _NC_BY_REPS[reps]


# ---------------------------------------------------------------- entry
def build_in_maps(inputs):
    import ml_dtypes

    bf16 = ml_dtypes.bfloat16
    query = np.asarray(inputs["query"], np.float32)
    key_in = np.asarray(inputs["key_in"], np.float32)
    value = np.asarray(inputs["value"], np.float32)
    Wq = np.asarray(inputs["Wq"], np.float32)
    Wk = np.asarray(inputs["Wk"], np.float32)
    Wv = np.asarray(inputs["Wv"], np.float32)
    bq = np.asarray(inputs["bq"], np.float32)
    bk = np.asarray(inputs["bk"], np.float32)
    bv = np.asarray(inputs["bv"], np.float32)

    in_maps = []
    for c in range(NCORES):
        b, hg = divmod(c, 2)
        cols = slice(hg * CPC, (hg + 1) * CPC)
        in_maps.append({
            "xqT": np.ascontiguousarray(query[b].T.astype(bf16)),
            "xkT": np.ascontiguousarray(key_in[b].T.astype(bf16)),
            "xvT": np.ascontiguousarray(value[b].T.astype(bf16)),
            "wq": np.ascontiguousarray(Wq[:, cols].astype(bf16)),
            "wk": np.ascontiguousarray(Wk[:, cols].astype(bf16)),
            "wv": np.ascontiguousarray(Wv[:, cols].astype(bf16)),
            "bq": np.ascontiguousarray(bq[cols]),
            "bk": np.ascontiguousarray(bk[cols]),
            "bv": np.ascontiguousarray(bv[cols]),
        })
    return in_maps


def kernel(query, key_in, value, Wq, bq, Wk, bk, Wv, bv):
    from concourse.bass_utils import run_bass_kernel_spmd

    nc = _get_nc()
    in_maps = build_in_maps({
        "query": query, "key_in": key_in, "value": value,
        "Wq": Wq, "bq": bq, "Wk": Wk, "bk": bk, "Wv": Wv, "bv": bv,
    })

    res = run_bass_kernel_spmd(nc, in_maps, core_ids=list(range(NCORES)))

    out = np.empty((B, S, D), np.float32)
    for c in range(NCORES):
        b, hg = divmod(c, 2)
        out[b, :, hg * CPC:(hg + 1) * CPC] = res.results[c]["out"].T
    return out
